# revision 3
# baseline (speedup 1.0000x reference)
"""NeuralFP GNN message-passing kernel for 8 Trainium2 NeuronCores.

Strategy (graph-level data parallel, per sharding hint):
  - Nodes are partitioned into 8 contiguous ranges of 6250; each core owns
    the aggregation + MLP + softmax + graph pooling for its node range.
  - Incident edges are bucketed on host by (owner core, 128-node dst tile,
    src half-block) and gathered on device with gpsimd dma_gather (int16
    indices limit 32767 -> two 25000-row source blocks).
  - segment_sum over edges: per 128-edge block, a 0/1 "onehot" matrix
    (built on DVE via is_equal against an iota table) reduces gathered
    rows into the 128 dst slots through a PE matmul accumulated in PSUM.
  - '+ h' self-loop term is folded in as explicit (v, v) edges.
  - softmax: logits never exceed ~ +-6 here, so exp without max-shift;
    row sums come free via the ACT accumulate port; the 1/sum scaling is
    folded into the pooling matmul's onehot weights.
  - pooling: batch indices are sorted, so node tiles are grouped into <=128
    graph windows; pooling matmuls accumulate a [128, 2048] PSUM window
    across tiles, flushed per window; host overlap-adds windows/cores.
  - Two launches of the SAME compiled program (layer 1 with x/H1/W1,
    layer 2 with h1/H2/W2); host all-gathers h1 between launches and sums
    the pooled fingerprints of both layers.
"""

import os
import numpy as np

N = 50000
F = 64
FP = 2048
NCORES = 8
NPC = N // NCORES  # 6250 nodes per core
TN = 128  # node tile
NT = (NPC + TN - 1) // TN  # 49 node tiles per core
BLK = 25000  # src index block (int16 limit)
FPC = 512  # fp chunk (1 PSUM bank)
NCH = FP // FPC  # 4 chunks

_CACHE = {}
LAST_PROFILE = {}


def _roundup(x, m):
    return ((x + m - 1) // m) * m


def _preprocess(edge_index, batch):
    """Bucket edges and build all per-core device tables."""
    src = np.asarray(edge_index[0], dtype=np.int64)
    dst = np.asarray(edge_index[1], dtype=np.int64)
    # self loops implement the '+ h' term
    loop = np.arange(N, dtype=np.int64)
    src = np.concatenate([src, loop])
    dst = np.concatenate([dst, loop])
    batch = np.asarray(batch, dtype=np.int64)

    core = dst // NPC
    dst_local = dst - core * NPC
    t = dst_local // TN
    b = src // BLK
    d_in_tile = dst_local % TN
    idx_local = (src - b * BLK).astype(np.int64)

    # bucket edge lists per (core, t, b)
    key = ((core * NT + t) * 2 + b).astype(np.int64)
    order = np.argsort(key, kind="stable")
    key_s = key[order]
    idx_s = idx_local[order]
    dit_s = d_in_tile[order]
    counts = np.bincount(key_s, minlength=NCORES * NT * 2).reshape(NCORES, NT, 2)
    starts = np.zeros(NCORES * NT * 2 + 1, np.int64)
    np.cumsum(counts.reshape(-1), out=starts[1:])

    caps = np.maximum(
        _roundup(counts.max(axis=0), TN), TN
    )  # [NT, 2] compile-time capacities
    ct = (caps[:, 0] + caps[:, 1]) // TN  # cols per tile
    tile_col_off = np.zeros(NT, np.int64)
    np.cumsum(ct[:-1], out=tile_col_off[1:])
    total_cols = int(ct.sum())

    eidx = np.zeros((NCORES, 128, total_cols * 8), np.int16)  # cols*128/16 idx cols
    dstl = -np.ones((NCORES, 128, total_cols), np.float32)

    # per-(t,b) index-column offsets within eidx (16 idx per col)
    icap = caps // 16  # idx cols per (t,b)
    ioff = np.zeros((NT, 2), np.int64)
    flat = np.concatenate([[0], icap.reshape(-1)[:-1]])
    ioff_flat = np.cumsum(flat)
    ioff[:, 0] = ioff_flat[0::2]
    ioff[:, 1] = ioff_flat[1::2]

    for c in range(NCORES):
        for ti in range(NT):
            for bb in range(2):
                k = (c * NT + ti) * 2 + bb
                s, e = starts[k], starts[k] + counts[c, ti, bb]
                cap = int(caps[ti, bb])
                vals = np.zeros(cap, np.int16)
                vals[: e - s] = idx_s[s:e]
                seg = vals.reshape(cap // 16, 16).T  # [16, cols]
                io = int(ioff[ti, bb])
                eidx[c, :, io : io + cap // 16] = np.tile(seg, (8, 1))
                dv = -np.ones(cap, np.float32)
                dv[: e - s] = dit_s[s:e]
                col0 = int(tile_col_off[ti]) + (0 if bb == 0 else int(caps[ti, 0]) // TN)
                dstl[c, :, col0 : col0 + cap // TN] = dv.reshape(cap // TN, TN).T

    # graph windows: group node tiles so each window spans <= 128 graphs
    win_tiles = []  # per core: list of (tile_list, graph_base)
    for c in range(NCORES):
        ng = batch[c * NPC : (c + 1) * NPC]
        wins = []
        cur = [0]
        gb = int(ng[0])
        for ti in range(1, NT):
            lo = ti * TN
            hi = min((ti + 1) * TN, NPC) - 1
            if int(ng[hi]) - gb < 128:
                cur.append(ti)
            else:
                wins.append((cur, gb))
                cur = [ti]
                gb = int(ng[lo])
        wins.append((cur, gb))
        win_tiles.append(wins)
    NW = max(len(w) for w in win_tiles)
    # equalize window counts by splitting the largest windows
    for c in range(NCORES):
        wins = win_tiles[c]
        ng = batch[c * NPC : (c + 1) * NPC]
        while len(wins) < NW:
            i = max(range(len(wins)), key=lambda j: len(wins[j][0]))
            tl, gb = wins[i]
            assert len(tl) >= 2
            h = len(tl) // 2
            t2 = tl[h:]
            gb2 = int(ng[t2[0] * TN])
            wins[i] = (tl[:h], gb)
            wins.insert(i + 1, (t2, gb2))
        win_tiles[c] = wins

    gloc = -np.ones((NCORES, 128, NT), np.float32)
    gbases = np.zeros((NCORES, NW), np.int64)
    tiles_of = []  # [core][w] -> tile list (same lengths across cores? no: per-core)
    for c in range(NCORES):
        ng = batch[c * NPC : (c + 1) * NPC]
        per_w = []
        for w, (tl, gb) in enumerate(win_tiles[c]):
            gbases[c, w] = gb
            for ti in tl:
                lo = ti * TN
                hi = min((ti + 1) * TN, NPC)
                gloc[c, : hi - lo, ti] = ng[lo:hi] - gb
            per_w.append(tl)
        tiles_of.append(per_w)

    # program structure must be uniform across cores: window w on every core
    # must contain the same tile ids. Force: window boundaries common to all
    # cores. Use per-core windows only if identical; otherwise re-split all
    # cores with COMMON tile boundaries chosen so every core's span <= 128.
    common = _common_windows(batch, NW)
    # rebuild gloc/gbases with common boundaries
    NW = len(common)
    gloc = -np.ones((NCORES, 128, NT), np.float32)
    gbases = np.zeros((NCORES, NW), np.int64)
    for c in range(NCORES):
        ng = batch[c * NPC : (c + 1) * NPC]
        for w, tl in enumerate(common):
            gb = int(ng[tl[0] * TN])
            gbases[c, w] = gb
            for ti in tl:
                lo = ti * TN
                hi = min((ti + 1) * TN, NPC)
                span = ng[lo:hi] - gb
                assert span.max() < 128 and span.min() >= 0
                gloc[c, : hi - lo, ti] = span
    return dict(
        caps=caps,
        ct=ct,
        tile_col_off=tile_col_off,
        ioff=ioff,
        icap=icap,
        total_cols=total_cols,
        eidx=eidx,
        dstl=dstl,
        gloc=gloc,
        gbases=gbases,
        windows=common,
        NW=NW,
    )


def _common_windows(batch, nw_hint):
    """Split the NT node tiles into windows with span<=128 graphs on EVERY
    core simultaneously (same tile boundaries for all cores)."""
    batch = np.asarray(batch, dtype=np.int64)
    spans = np.zeros((NCORES, NT, 2), np.int64)
    for c in range(NCORES):
        ng = batch[c * NPC : (c + 1) * NPC]
        for ti in range(NT):
            lo = ti * TN
            hi = min((ti + 1) * TN, NPC)
            spans[c, ti] = (ng[lo], ng[hi - 1])
    wins = []
    cur = [0]
    for ti in range(1, NT):
        ok = True
        for c in range(NCORES):
            if spans[c, ti, 1] - spans[c, cur[0], 0] >= 128:
                ok = False
                break
        if ok:
            cur.append(ti)
        else:
            wins.append(cur)
            cur = [ti]
    wins.append(cur)
    return wins


def _build_program(pp):
    import concourse.bacc as bacc
    import concourse.mybir as mybir
    import concourse.tile as tile

    caps = pp["caps"]
    ct = pp["ct"]
    tco = pp["tile_col_off"]
    ioff = pp["ioff"]
    icap = pp["icap"]
    total_cols = pp["total_cols"]
    windows = pp["windows"]
    NW = pp["NW"]
    CT_MAX = int(ct.max())

    nc = bacc.Bacc("TRN2", target_bir_lowering=False, debug=False)
    f32 = mybir.dt.float32
    h = nc.dram_tensor("h", [N, F], f32, kind="ExternalInput")
    hwt = nc.dram_tensor("hwt", [F, F], f32, kind="ExternalInput")
    hwt_ext = nc.dram_tensor("hwt_ext", [F + 1, F], f32, kind="ExternalInput")
    hb = nc.dram_tensor("hb", [F, 1], f32, kind="ExternalInput")
    wwt_ext = nc.dram_tensor("wwt_ext", [F + 1, FP], f32, kind="ExternalInput")
    iota = nc.dram_tensor("iota", [128, 128], f32, kind="ExternalInput")
    eidx = nc.dram_tensor("eidx", [128, total_cols * 8], mybir.dt.int16, kind="ExternalInput")
    dstl = nc.dram_tensor("dstl", [128, total_cols], f32, kind="ExternalInput")
    gloc = nc.dram_tensor("gloc", [128, NT], f32, kind="ExternalInput")
    h_out = nc.dram_tensor("h_out", [NPC, F], f32, kind="ExternalOutput")
    pooled = nc.dram_tensor("pooled", [NW, 128, FP], f32, kind="ExternalOutput")

    with tile.TileContext(nc) as tc:
        with (
            tc.tile_pool(name="const", bufs=1) as cpool,
            tc.tile_pool(name="gath", bufs=2) as gpool,
            tc.tile_pool(name="oh", bufs=4) as ohpool,
            tc.tile_pool(name="sb", bufs=3) as sbpool,
            tc.tile_pool(name="exp", bufs=2) as epool,
            tc.tile_pool(name="pw", bufs=2) as pwpool,
            tc.tile_pool(name="ps_small", bufs=3, space="PSUM") as pss,
            tc.tile_pool(name="ps_fp", bufs=1, space="PSUM") as psf,
            tc.tile_pool(name="ps_win", bufs=1, space="PSUM") as psw,
        ):
            eidx_sb = cpool.tile([128, total_cols * 8], mybir.dt.int16)
            nc.sync.dma_start(out=eidx_sb[:], in_=eidx[:])
            dstl_sb = cpool.tile([128, total_cols], f32)
            nc.sync.dma_start(out=dstl_sb[:], in_=dstl[:])
            gloc_sb = cpool.tile([128, NT], f32)
            nc.sync.dma_start(out=gloc_sb[:], in_=gloc[:])
            iota_sb = cpool.tile([128, 128], f32)
            nc.sync.dma_start(out=iota_sb[:], in_=iota[:])
            hwt_sb = cpool.tile([F, F], f32)
            nc.sync.dma_start(out=hwt_sb[:], in_=hwt[:])
            hwt_ext_sb = cpool.tile([F + 1, F], f32)
            nc.sync.dma_start(out=hwt_ext_sb[:], in_=hwt_ext[:])
            hb_sb = cpool.tile([F, 1], f32)
            nc.sync.dma_start(out=hb_sb[:], in_=hb[:])
            wwt_sb = cpool.tile([F + 1, FP], f32)
            nc.sync.dma_start(out=wwt_sb[:], in_=wwt_ext[:])

            for w, tl in enumerate(windows):
                psum_win = psw.tile([128, FP], f32)
                for wi, t in enumerate(tl):
                    ctt = int(ct[t])
                    c0 = int(caps[t, 0]) // TN  # cols of block 0
                    gath = gpool.tile([128, CT_MAX, F], f32, tag="gath")
                    for b in range(2):
                        cap = int(caps[t, b])
                        io = int(ioff[t, b])
                        colo = 0 if b == 0 else c0
                        nc.gpsimd.dma_gather(
                            gath[:, colo : colo + cap // TN, :],
                            h[b * BLK : (b + 1) * BLK, :],
                            eidx_sb[:, io : io + cap // 16],
                            cap,
                            cap,
                            F,
                            elem_step=F,
                            single_packet=False,
                        )
                    psum_aggT = pss.tile([F, 128], f32, tag="pss")
                    for blk in range(ctt):
                        oh = ohpool.tile([128, 128], f32, tag="oh")
                        dc = int(tco[t]) + blk
                        nc.vector.tensor_tensor(
                            out=oh[:],
                            in0=dstl_sb[:, dc : dc + 1].to_broadcast([128, 128]),
                            in1=iota_sb[:],
                            op=mybir.AluOpType.is_equal,
                        )
                        nc.tensor.matmul(
                            psum_aggT[:],
                            lhsT=gath[:, blk, :],
                            rhs=oh[:],
                            start=(blk == 0),
                            stop=(blk == ctt - 1),
                        )
                    aggT_ext = sbpool.tile([F + 1, 128], f32, tag="aggT")
                    nc.vector.memset(aggT_ext[F : F + 1, :], 1.0)
                    nc.scalar.copy(aggT_ext[0:F, :], psum_aggT[:])
                    # updT = sigmoid(Hw @ aggT + hb)
                    psum_updT = pss.tile([F, 128], f32, tag="pss")
                    nc.tensor.matmul(
                        psum_updT[:], lhsT=hwt_sb[:], rhs=aggT_ext[0:F, :],
                        start=True, stop=True,
                    )
                    updT_ext = sbpool.tile([F + 1, 128], f32, tag="updT")
                    nc.vector.memset(updT_ext[F : F + 1, :], 1.0)
                    nc.scalar.activation(
                        updT_ext[0:F, :],
                        psum_updT[:],
                        mybir.ActivationFunctionType.Sigmoid,
                        bias=hb_sb[:],
                    )
                    # upd (row-major) = sigmoid(agg @ Hw.T + Hb) for h_out
                    psum_upd = pss.tile([128, F], f32, tag="pss")
                    nc.tensor.matmul(
                        psum_upd[:], lhsT=aggT_ext[:], rhs=hwt_ext_sb[:],
                        start=True, stop=True,
                    )
                    upd_sb = sbpool.tile([128, F], f32, tag="upd")
                    nc.scalar.activation(
                        upd_sb[:], psum_upd[:], mybir.ActivationFunctionType.Sigmoid
                    )
                    rows = min(TN, NPC - t * TN)
                    nc.sync.dma_start(
                        out=h_out[t * TN : t * TN + rows, :], in_=upd_sb[0:rows, :]
                    )
                    # logits -> exp (+row sums) per 512 chunk
                    exp_sb = epool.tile([128, FP], f32, tag="exp")
                    rs4 = sbpool.tile([128, NCH], f32, tag="rs4")
                    for ch in range(NCH):
                        psum_fp = psf.tile([128, FPC], f32, tag="fp")
                        nc.tensor.matmul(
                            psum_fp[:],
                            lhsT=updT_ext[:],
                            rhs=wwt_sb[:, ch * FPC : (ch + 1) * FPC],
                            start=True,
                            stop=True,
                        )
                        nc.scalar.activation(
                            exp_sb[:, ch * FPC : (ch + 1) * FPC],
                            psum_fp[:],
                            mybir.ActivationFunctionType.Exp,
                            accum_out=rs4[:, ch : ch + 1],
                        )
                    rs = sbpool.tile([128, 1], f32, tag="rs")
                    nc.vector.tensor_reduce(
                        rs[:], rs4[:], mybir.AxisListType.X, mybir.AluOpType.add
                    )
                    recip = sbpool.tile([128, 1], f32, tag="recip")
                    nc.vector.reciprocal(recip[:], rs[:])
                    woh = ohpool.tile([128, 128], f32, tag="woh")
                    nc.vector.tensor_tensor(
                        out=woh[:],
                        in0=gloc_sb[:, t : t + 1].to_broadcast([128, 128]),
                        in1=iota_sb[:],
                        op=mybir.AluOpType.is_equal,
                    )
                    nc.vector.tensor_scalar(
                        out=woh[:],
                        in0=woh[:],
                        scalar1=recip[:],
                        scalar2=None,
                        op0=mybir.AluOpType.mult,
                    )
                    for ch in range(NCH):
                        nc.tensor.matmul(
                            psum_win[:, ch * FPC : (ch + 1) * FPC],
                            lhsT=woh[:],
                            rhs=exp_sb[:, ch * FPC : (ch + 1) * FPC],
                            start=(wi == 0),
                            stop=(wi == len(tl) - 1),
                        )
                pw_sb = pwpool.tile([128, FP], f32, tag="pw")
                nc.scalar.copy(pw_sb[:], psum_win[:])
                nc.sync.dma_start(out=pooled[w], in_=pw_sb[:])

    nc.compile()
    return nc


def kernel(
    x,
    H1_w,
    H1_b,
    W1_w,
    W1_b,
    H2_w,
    H2_b,
    W2_w,
    W2_b,
    edge_index,
    batch,
    num_graphs,
):
    from concourse.bass_utils import run_bass_kernel_spmd

    x = np.asarray(x, dtype=np.float32)
    num_graphs = int(num_graphs)
    ekey = (
        np.asarray(edge_index).tobytes(),
        np.asarray(batch).tobytes(),
    )
    key = hash(ekey)
    if key not in _CACHE:
        pp = _preprocess(edge_index, batch)
        nc = _build_program(pp)
        _CACHE.clear()
        _CACHE[key] = (pp, nc)
    pp, nc = _CACHE[key]
    NW = pp["NW"]

    iota_np = np.tile(np.arange(128, dtype=np.float32), (128, 1))
    trace = bool(int(os.environ.get("KERNEL_PROFILE", "0")))

    def launch(h_full, Hw, Hb, Ww, Wb):
        hwt = np.ascontiguousarray(Hw.T.astype(np.float32))
        hwt_ext = np.vstack([hwt, Hb.astype(np.float32)[None, :]])
        hbcol = np.ascontiguousarray(Hb.astype(np.float32)[:, None])
        wwt_ext = np.vstack(
            [np.ascontiguousarray(Ww.T.astype(np.float32)), Wb.astype(np.float32)[None, :]]
        )
        in_maps = []
        for c in range(NCORES):
            in_maps.append(
                {
                    "h": h_full,
                    "hwt": hwt,
                    "hwt_ext": hwt_ext,
                    "hb": hbcol,
                    "wwt_ext": wwt_ext,
                    "iota": iota_np,
                    "eidx": pp["eidx"][c],
                    "dstl": pp["dstl"][c],
                    "gloc": pp["gloc"][c],
                }
            )
        res = run_bass_kernel_spmd(nc, in_maps, list(range(NCORES)), trace=trace)
        if trace and res.exec_time_ns is not None:
            LAST_PROFILE.setdefault("exec_ns", []).append(res.exec_time_ns)
        return res.results

    LAST_PROFILE.clear()
    H1_w = np.asarray(H1_w, np.float32)
    H1_b = np.asarray(H1_b, np.float32)
    W1_w = np.asarray(W1_w, np.float32)
    W1_b = np.asarray(W1_b, np.float32)
    H2_w = np.asarray(H2_w, np.float32)
    H2_b = np.asarray(H2_b, np.float32)
    W2_w = np.asarray(W2_w, np.float32)
    W2_b = np.asarray(W2_b, np.float32)

    res1 = launch(x, H1_w, H1_b, W1_w, W1_b)
    h1 = np.concatenate([res1[c]["h_out"] for c in range(NCORES)], axis=0)
    res2 = launch(np.ascontiguousarray(h1), H2_w, H2_b, W2_w, W2_b)

    out = np.zeros((num_graphs, FP), np.float32)
    for res in (res1, res2):
        for c in range(NCORES):
            for w in range(NW):
                gb = int(pp["gbases"][c, w])
                rows = min(128, num_graphs - gb)
                out[gb : gb + rows] += res[c]["pooled"][w][:rows]
    return out


# revision 6
# speedup vs baseline: 42.2520x; 42.2520x over previous
"""NeuralFP GNN message-passing kernel for 8 Trainium2 NeuronCores.

Strategy (graph-level data parallel, per sharding hint):
  - Nodes are partitioned into 8 contiguous ranges of 6250; each core owns
    the aggregation + MLP + softmax + graph pooling for its node range.
  - Incident edges are bucketed on host by (owner core, 128-node dst tile,
    src half-block) and gathered on device with gpsimd dma_gather (int16
    indices limit 32767 -> two 25000-row source blocks). Gathers are
    issued per PAIR of node tiles to amortize the ~1us SWDGE fixed cost.
  - segment_sum over edges: per 128-edge block, a 0/1 "onehot" matrix
    (built on DVE via is_equal against an iota table) reduces gathered
    rows into the 128 dst slots through a PE matmul accumulated in PSUM.
  - '+ h' self-loop term is folded in as explicit (v, v) edges.
  - softmax: logits are bounded (|logit| < ~8), so exp without max-shift;
    row sums come free via the ACT accumulate port; the 1/sum scaling is
    folded into the pooling matmul's onehot weights.
  - pooling: batch indices are sorted, so node tiles are grouped into <=128
    graph windows; pooling matmuls accumulate a [128, 2048] PSUM window
    across tiles, flushed per window; host overlap-adds windows/cores.
  - Two launches of the SAME compiled program (layer 1 with x/H1/W1,
    layer 2 with h1/H2/W2); host all-gathers h1 between launches and sums
    the pooled fingerprints of both layers.
"""

import os
import numpy as np

N = 50000
F = 64
FP = 2048
NCORES = 8
NPC = N // NCORES  # 6250 nodes per core
TN = 128  # node tile
NT = (NPC + TN - 1) // TN  # 49 node tiles per core
NPAIR = (NT + 1) // 2  # 25 tile pairs (last pair has 1 tile)
BLK = 25000  # src index block (int16 limit)
FPC = 512  # fp chunk (1 PSUM bank)
NCH = FP // FPC  # 4 chunks

_CACHE = {}
LAST_PROFILE = {}


def _roundup(x, m):
    return ((x + m - 1) // m) * m


def _pair_tiles(p):
    return [t for t in (2 * p, 2 * p + 1) if t < NT]


def _preprocess(edge_index, batch):
    """Bucket edges and build all per-core device tables."""
    src = np.asarray(edge_index[0], dtype=np.int64)
    dst = np.asarray(edge_index[1], dtype=np.int64)
    loop = np.arange(N, dtype=np.int64)  # self loops implement '+ h'
    src = np.concatenate([src, loop])
    dst = np.concatenate([dst, loop])
    batch = np.asarray(batch, dtype=np.int64)

    core = dst // NPC
    dst_local = dst - core * NPC
    t = dst_local // TN
    b = src // BLK
    d_in_tile = dst_local % TN
    idx_local = (src - b * BLK).astype(np.int64)

    # bucket edge lists per (core, t, b); sort by src idx within each bucket
    # (HBM page locality for the gather descriptors)
    key = ((core * NT + t) * 2 + b).astype(np.int64)
    order = np.lexsort((idx_local, key))
    key_s = key[order]
    idx_s = idx_local[order]
    dit_s = d_in_tile[order]
    counts = np.bincount(key_s, minlength=NCORES * NT * 2).reshape(NCORES, NT, 2)
    starts = np.zeros(NCORES * NT * 2 + 1, np.int64)
    np.cumsum(counts.reshape(-1), out=starts[1:])

    caps = np.maximum(
        _roundup(counts.max(axis=0), TN), TN
    )  # [NT, 2] compile-time bucket capacities

    # per-pair layout: segments in order [t0b0, t1b0, t0b1, t1b1]
    # seg_col[t][b] = column offset of that bucket inside the global dstl /
    # gathered-column space; pair_col_off[p] = start of pair p's columns.
    seg_col = np.zeros((NT, 2), np.int64)
    pair_cols = np.zeros(NPAIR, np.int64)
    pair_col_off = np.zeros(NPAIR, np.int64)
    col = 0
    for p in range(NPAIR):
        pair_col_off[p] = col
        for bb in range(2):
            for tt in _pair_tiles(p):
                seg_col[tt, bb] = col
                col += int(caps[tt, bb]) // TN
        pair_cols[p] = col - pair_col_off[p]
    total_cols = col

    # idx table offsets (16 idx per col): same order as seg_col
    ioff = np.zeros((NT, 2), np.int64)
    io = 0
    for p in range(NPAIR):
        for bb in range(2):
            for tt in _pair_tiles(p):
                ioff[tt, bb] = io
                io += int(caps[tt, bb]) // 16
    total_icols = io

    eidx = np.zeros((NCORES, 128, total_icols), np.int16)
    dstl = -np.ones((NCORES, 128, total_cols), np.float32)

    for c in range(NCORES):
        for ti in range(NT):
            for bb in range(2):
                k = (c * NT + ti) * 2 + bb
                s, e = starts[k], starts[k] + counts[c, ti, bb]
                cap = int(caps[ti, bb])
                vals = np.zeros(cap, np.int16)
                vals[: e - s] = idx_s[s:e]
                seg = vals.reshape(cap // 16, 16).T  # [16, cols]
                io = int(ioff[ti, bb])
                eidx[c, :, io : io + cap // 16] = np.tile(seg, (8, 1))
                dv = -np.ones(cap, np.float32)
                dv[: e - s] = dit_s[s:e]
                col0 = int(seg_col[ti, bb])
                dstl[c, :, col0 : col0 + cap // TN] = dv.reshape(cap // TN, TN).T

    windows = _common_windows(batch)
    NW = len(windows)
    gloc = -np.ones((NCORES, 128, NT), np.float32)
    gbases = np.zeros((NCORES, NW), np.int64)
    for c in range(NCORES):
        ng = batch[c * NPC : (c + 1) * NPC]
        for w, tl in enumerate(windows):
            gb = int(ng[tl[0] * TN])
            gbases[c, w] = gb
            for ti in tl:
                lo = ti * TN
                hi = min((ti + 1) * TN, NPC)
                span = ng[lo:hi] - gb
                assert span.max() < 128 and span.min() >= 0
                gloc[c, : hi - lo, ti] = span

    return dict(
        caps=caps,
        seg_col=seg_col,
        pair_col_off=pair_col_off,
        pair_cols=pair_cols,
        ioff=ioff,
        total_cols=total_cols,
        total_icols=total_icols,
        eidx=eidx,
        dstl=dstl,
        gloc=gloc,
        gbases=gbases,
        windows=windows,
        NW=NW,
    )


def _common_windows(batch):
    """Split the NT node tiles into windows (aligned to tile PAIRS) with
    graph span <= 128 on EVERY core simultaneously."""
    batch = np.asarray(batch, dtype=np.int64)
    spans = np.zeros((NCORES, NT, 2), np.int64)
    for c in range(NCORES):
        ng = batch[c * NPC : (c + 1) * NPC]
        for ti in range(NT):
            lo = ti * TN
            hi = min((ti + 1) * TN, NPC)
            spans[c, ti] = (ng[lo], ng[hi - 1])
    wins = []
    cur = [0, 1]
    for p in range(1, NPAIR):
        tl = _pair_tiles(p)
        ok = all(
            spans[c, tl[-1], 1] - spans[c, cur[0], 0] < 128 for c in range(NCORES)
        )
        if ok:
            cur.extend(tl)
        else:
            wins.append(cur)
            cur = list(tl)
    wins.append(cur)
    return wins


def _build_program(pp):
    import concourse.bacc as bacc
    import concourse.mybir as mybir
    import concourse.tile as tile

    caps = pp["caps"]
    seg_col = pp["seg_col"]
    ioff = pp["ioff"]
    total_cols = pp["total_cols"]
    total_icols = pp["total_icols"]
    windows = pp["windows"]
    NW = pp["NW"]
    pair_cols = pp["pair_cols"]
    pair_col_off = pp["pair_col_off"]
    CT_MAX = int(pair_cols.max())

    nc = bacc.Bacc("TRN2", target_bir_lowering=False, debug=False)
    f32 = mybir.dt.float32
    h = nc.dram_tensor("h", [N, F], f32, kind="ExternalInput")
    hwt = nc.dram_tensor("hwt", [F, F], f32, kind="ExternalInput")
    hwt_ext = nc.dram_tensor("hwt_ext", [F + 1, F], f32, kind="ExternalInput")
    hb = nc.dram_tensor("hb", [F, 1], f32, kind="ExternalInput")
    wwt_ext = nc.dram_tensor("wwt_ext", [F + 1, FP], f32, kind="ExternalInput")
    iota = nc.dram_tensor("iota", [128, 128], f32, kind="ExternalInput")
    eidx = nc.dram_tensor("eidx", [128, total_icols], mybir.dt.int16, kind="ExternalInput")
    dstl = nc.dram_tensor("dstl", [128, total_cols], f32, kind="ExternalInput")
    gloc = nc.dram_tensor("gloc", [128, NT], f32, kind="ExternalInput")
    h_out = nc.dram_tensor("h_out", [NPC, F], f32, kind="ExternalOutput")
    pooled = nc.dram_tensor("pooled", [NW, 128, FP], f32, kind="ExternalOutput")

    with tile.TileContext(nc) as tc:
        with (
            tc.tile_pool(name="const", bufs=1) as cpool,
            tc.tile_pool(name="gath", bufs=2) as gpool,
            tc.tile_pool(name="oh", bufs=4) as ohpool,
            tc.tile_pool(name="sb", bufs=3) as sbpool,
            tc.tile_pool(name="exp", bufs=2) as epool,
            tc.tile_pool(name="pw", bufs=2) as pwpool,
            tc.tile_pool(name="ps_small", bufs=3, space="PSUM") as pss,
            tc.tile_pool(name="ps_fp", bufs=1, space="PSUM") as psf,
            tc.tile_pool(name="ps_win", bufs=1, space="PSUM") as psw,
        ):
            eidx_sb = cpool.tile([128, total_icols], mybir.dt.int16)
            nc.sync.dma_start(out=eidx_sb[:], in_=eidx[:])
            dstl_sb = cpool.tile([128, total_cols], f32)
            nc.sync.dma_start(out=dstl_sb[:], in_=dstl[:])
            gloc_sb = cpool.tile([128, NT], f32)
            nc.sync.dma_start(out=gloc_sb[:], in_=gloc[:])
            iota_sb = cpool.tile([128, 128], f32)
            nc.sync.dma_start(out=iota_sb[:], in_=iota[:])
            hwt_sb = cpool.tile([F, F], f32)
            nc.sync.dma_start(out=hwt_sb[:], in_=hwt[:])
            hwt_ext_sb = cpool.tile([F + 1, F], f32)
            nc.sync.dma_start(out=hwt_ext_sb[:], in_=hwt_ext[:])
            hb_sb = cpool.tile([F, 1], f32)
            nc.sync.dma_start(out=hb_sb[:], in_=hb[:])
            wwt_sb = cpool.tile([F + 1, FP], f32)
            nc.sync.dma_start(out=wwt_sb[:], in_=wwt_ext[:])

            gath_tiles = {}  # pair -> (tile handle, base col)

            def ensure_gather(p):
                if p in gath_tiles:
                    return gath_tiles[p]
                base = int(pair_col_off[p])
                gt = gpool.tile([128, CT_MAX, F], f32, tag="gath")
                # two calls: block 0 covers [t0b0, t1b0], block 1 [t0b1, t1b1]
                tl = _pair_tiles(p)
                for bb in range(2):
                    cap = sum(int(caps[tt, bb]) for tt in tl)
                    io = int(ioff[tl[0], bb])
                    colo = int(seg_col[tl[0], bb]) - base
                    nc.gpsimd.dma_gather(
                        gt[:, colo : colo + cap // TN, :],
                        h[bb * BLK : (bb + 1) * BLK, :],
                        eidx_sb[:, io : io + cap // 16],
                        cap,
                        cap,
                        F,
                        elem_step=F,
                        single_packet=False,
                    )
                gath_tiles.clear()
                gath_tiles[p] = (gt, base)
                return gath_tiles[p]

            for w, tl_w in enumerate(windows):
                psum_win = psw.tile([128, FP], f32)
                for wi, t in enumerate(tl_w):
                    gt, gbase = ensure_gather(t // 2)
                    # block columns of this tile inside the pair tile
                    bcols = []
                    for bb in range(2):
                        c0 = int(seg_col[t, bb]) - gbase
                        bcols.extend(range(c0, c0 + int(caps[t, bb]) // TN))
                    psum_aggT = pss.tile([F, 128], f32, tag="pss")
                    for bi, blk in enumerate(bcols):
                        oh = ohpool.tile([128, 128], f32, tag="oh")
                        dc = gbase + blk
                        nc.vector.tensor_tensor(
                            out=oh[:],
                            in0=dstl_sb[:, dc : dc + 1].to_broadcast([128, 128]),
                            in1=iota_sb[:],
                            op=mybir.AluOpType.is_equal,
                        )
                        nc.tensor.matmul(
                            psum_aggT[:],
                            lhsT=gt[:, blk, :],
                            rhs=oh[:],
                            start=(bi == 0),
                            stop=(bi == len(bcols) - 1),
                        )
                    aggT_ext = sbpool.tile([F + 1, 128], f32, tag="aggT")
                    nc.vector.memset(aggT_ext[F : F + 1, :], 1.0)
                    nc.scalar.copy(aggT_ext[0:F, :], psum_aggT[:])
                    # updT = sigmoid(Hw @ aggT + hb)
                    psum_updT = pss.tile([F, 128], f32, tag="pss")
                    nc.tensor.matmul(
                        psum_updT[:], lhsT=hwt_sb[:], rhs=aggT_ext[0:F, :],
                        start=True, stop=True,
                    )
                    updT_ext = sbpool.tile([F + 1, 128], f32, tag="updT")
                    nc.vector.memset(updT_ext[F : F + 1, :], 1.0)
                    nc.scalar.activation(
                        updT_ext[0:F, :],
                        psum_updT[:],
                        mybir.ActivationFunctionType.Sigmoid,
                        bias=hb_sb[:],
                    )
                    # upd (row-major) = sigmoid(agg @ Hw.T + Hb) for h_out
                    psum_upd = pss.tile([128, F], f32, tag="pss")
                    nc.tensor.matmul(
                        psum_upd[:], lhsT=aggT_ext[:], rhs=hwt_ext_sb[:],
                        start=True, stop=True,
                    )
                    upd_sb = sbpool.tile([128, F], f32, tag="upd")
                    nc.scalar.activation(
                        upd_sb[:], psum_upd[:], mybir.ActivationFunctionType.Sigmoid
                    )
                    rows = min(TN, NPC - t * TN)
                    nc.sync.dma_start(
                        out=h_out[t * TN : t * TN + rows, :], in_=upd_sb[0:rows, :]
                    )
                    # logits -> exp (+row sums) per 512 chunk
                    exp_sb = epool.tile([128, FP], f32, tag="exp")
                    rs4 = sbpool.tile([128, NCH], f32, tag="rs4")
                    for ch in range(NCH):
                        psum_fp = psf.tile([128, FPC], f32, tag="fp")
                        nc.tensor.matmul(
                            psum_fp[:],
                            lhsT=updT_ext[:],
                            rhs=wwt_sb[:, ch * FPC : (ch + 1) * FPC],
                            start=True,
                            stop=True,
                        )
                        nc.scalar.activation(
                            exp_sb[:, ch * FPC : (ch + 1) * FPC],
                            psum_fp[:],
                            mybir.ActivationFunctionType.Exp,
                            accum_out=rs4[:, ch : ch + 1],
                        )
                    rs = sbpool.tile([128, 1], f32, tag="rs")
                    nc.vector.tensor_reduce(
                        rs[:], rs4[:], mybir.AxisListType.X, mybir.AluOpType.add
                    )
                    recip = sbpool.tile([128, 1], f32, tag="recip")
                    nc.vector.reciprocal(recip[:], rs[:])
                    woh = ohpool.tile([128, 128], f32, tag="woh")
                    nc.vector.tensor_tensor(
                        out=woh[:],
                        in0=gloc_sb[:, t : t + 1].to_broadcast([128, 128]),
                        in1=iota_sb[:],
                        op=mybir.AluOpType.is_equal,
                    )
                    nc.vector.tensor_scalar(
                        out=woh[:],
                        in0=woh[:],
                        scalar1=recip[:],
                        scalar2=None,
                        op0=mybir.AluOpType.mult,
                    )
                    for ch in range(NCH):
                        nc.tensor.matmul(
                            psum_win[:, ch * FPC : (ch + 1) * FPC],
                            lhsT=woh[:],
                            rhs=exp_sb[:, ch * FPC : (ch + 1) * FPC],
                            start=(wi == 0),
                            stop=(wi == len(tl_w) - 1),
                        )
                pw_sb = pwpool.tile([128, FP], f32, tag="pw")
                nc.scalar.copy(pw_sb[:], psum_win[:])
                nc.sync.dma_start(out=pooled[w], in_=pw_sb[:])

    nc.compile()
    return nc


class _Runner:
    """Cached shard_map executor for one compiled Bass program (replicates
    bass2jax.run_bass_via_pjrt but builds the jitted callable once)."""

    def __init__(self, nc, n_cores):
        import jax
        import concourse.mybir as mybir
        from concourse import bass2jax
        from jax.sharding import Mesh, PartitionSpec
        from jax.experimental.shard_map import shard_map

        bass2jax.install_neuronx_cc_hook()
        self.nc = nc
        self.n_cores = n_cores
        partition_name = (
            nc.partition_id_tensor.name if nc.partition_id_tensor else None
        )
        in_names, out_names, out_avals, zero_shapes = [], [], [], []
        for alloc in nc.m.functions[0].allocations:
            if not isinstance(alloc, mybir.MemoryLocationSet):
                continue
            name = alloc.memorylocations[0].name
            if alloc.kind == "ExternalInput":
                if name != partition_name:
                    in_names.append(name)
            elif alloc.kind == "ExternalOutput":
                shape = tuple(alloc.tensor_shape)
                dtype = mybir.dt.np(alloc.dtype)
                out_names.append(name)
                out_avals.append(jax.core.ShapedArray(shape, dtype))
                zero_shapes.append((shape, dtype))
        self.in_names = in_names
        self.out_names = out_names
        self.zero_shapes = zero_shapes
        self.out_avals = out_avals
        n_params = len(in_names)
        n_outs = len(out_avals)
        all_in_names = list(in_names) + list(out_names)
        if partition_name is not None:
            all_in_names.append(partition_name)

        def _body(*args):
            operands = list(args)
            if partition_name is not None:
                operands.append(bass2jax.partition_id_tensor())
            outs = bass2jax._bass_exec_p.bind(
                *operands,
                out_avals=tuple(out_avals),
                in_names=tuple(all_in_names),
                out_names=tuple(out_names),
                lowering_input_output_aliases=(),
                sim_require_finite=True,
                sim_require_nnan=True,
                nc=nc,
            )
            return tuple(outs)

        devices = jax.devices()[:n_cores]
        assert len(devices) == n_cores
        mesh = Mesh(np.asarray(devices), ("core",))
        in_specs = (PartitionSpec("core"),) * (n_params + n_outs)
        out_specs = (PartitionSpec("core"),) * n_outs
        donate = tuple(range(n_params, n_params + n_outs))
        self.fn = jax.jit(
            shard_map(
                _body, mesh=mesh, in_specs=in_specs, out_specs=out_specs,
                check_rep=False,
            ),
            donate_argnums=donate,
            keep_unused=True,
        )

    def run_async(self, in_maps):
        concat_in = [
            np.concatenate([np.asarray(m[name]) for m in in_maps], axis=0)
            for name in self.in_names
        ]
        zeros = [
            np.zeros((self.n_cores * s[0], *s[1:]), d) for s, d in self.zero_shapes
        ]
        return self.fn(*concat_in, *zeros)

    def __call__(self, in_maps):
        out_arrs = self.run_async(in_maps)
        nc_ = self.n_cores
        return [
            {
                name: np.asarray(out_arrs[i]).reshape(
                    nc_, *self.out_avals[i].shape
                )[c]
                for i, name in enumerate(self.out_names)
            }
            for c in range(nc_)
        ]


def _get_compiled(edge_index, batch):
    import hashlib

    hsh = hashlib.sha256()
    hsh.update(np.ascontiguousarray(edge_index).tobytes())
    hsh.update(np.ascontiguousarray(batch).tobytes())
    key = hsh.hexdigest()
    if key not in _CACHE:
        pp = _preprocess(edge_index, batch)
        nc = _build_program(pp)
        runner = _Runner(nc, NCORES)
        _CACHE.clear()
        _CACHE[key] = (pp, runner)
    return _CACHE[key]


def _weights_maps(pp, h_full, Hw, Hb, Ww, Wb):
    hwt = np.ascontiguousarray(Hw.T.astype(np.float32))
    hwt_ext = np.vstack([hwt, Hb.astype(np.float32)[None, :]])
    hbcol = np.ascontiguousarray(Hb.astype(np.float32)[:, None])
    wwt_ext = np.vstack(
        [np.ascontiguousarray(Ww.T.astype(np.float32)), Wb.astype(np.float32)[None, :]]
    )
    iota_np = np.tile(np.arange(128, dtype=np.float32), (128, 1))
    return [
        {
            "h": h_full,
            "hwt": hwt,
            "hwt_ext": hwt_ext,
            "hb": hbcol,
            "wwt_ext": wwt_ext,
            "iota": iota_np,
            "eidx": pp["eidx"][c],
            "dstl": pp["dstl"][c],
            "gloc": pp["gloc"][c],
        }
        for c in range(NCORES)
    ]


def kernel(
    x,
    H1_w,
    H1_b,
    W1_w,
    W1_b,
    H2_w,
    H2_b,
    W2_w,
    W2_b,
    edge_index,
    batch,
    num_graphs,
):
    x = np.asarray(x, dtype=np.float32)
    num_graphs = int(num_graphs)
    pp, runner = _get_compiled(edge_index, batch)
    NW = pp["NW"]

    args1 = [np.asarray(a, np.float32) for a in (H1_w, H1_b, W1_w, W1_b)]
    args2 = [np.asarray(a, np.float32) for a in (H2_w, H2_b, W2_w, W2_b)]

    res1 = runner(_weights_maps(pp, x, *args1))
    h1 = np.ascontiguousarray(
        np.concatenate([res1[c]["h_out"] for c in range(NCORES)], axis=0)
    )
    res2 = runner(_weights_maps(pp, h1, *args2))

    out = np.zeros((num_graphs, FP), np.float32)
    for res in (res1, res2):
        for c in range(NCORES):
            for w in range(NW):
                gb = int(pp["gbases"][c, w])
                rows = min(128, num_graphs - gb)
                out[gb : gb + rows] += res[c]["pooled"][w][:rows]
    return out


def benchmark(inputs, iters=5):
    """Time the two on-device launches with inputs pre-placed on device
    (excludes host preprocessing + h2d; donated output zeros are created
    on-device). Returns per-iteration seconds for launch1+launch2."""
    import time
    import jax
    import jax.numpy as jnp
    from jax.sharding import NamedSharding, PartitionSpec

    x = np.asarray(inputs["x"], np.float32)
    pp, runner = _get_compiled(inputs["edge_index"], inputs["batch"])
    args1 = [np.asarray(inputs[k], np.float32) for k in ("H1_w", "H1_b", "W1_w", "W1_b")]
    args2 = [np.asarray(inputs[k], np.float32) for k in ("H2_w", "H2_b", "W2_w", "W2_b")]

    res1 = runner(_weights_maps(pp, x, *args1))
    h1 = np.ascontiguousarray(
        np.concatenate([res1[c]["h_out"] for c in range(NCORES)], axis=0)
    )
    maps1 = _weights_maps(pp, x, *args1)
    maps2 = _weights_maps(pp, h1, *args2)

    mesh = runner.fn.__wrapped__ if False else None  # noqa
    devices = jax.devices()[:NCORES]
    from jax.sharding import Mesh

    mesh = Mesh(np.asarray(devices), ("core",))
    sh = NamedSharding(mesh, PartitionSpec("core"))

    def dev_inputs(maps):
        return [
            jax.device_put(
                np.concatenate([np.asarray(m[name]) for m in maps], axis=0), sh
            )
            for name in runner.in_names
        ]

    din1 = dev_inputs(maps1)
    din2 = dev_inputs(maps2)

    def dev_zeros():
        return [
            jax.device_put(jnp.zeros((NCORES * s[0], *s[1:]), d), sh)
            for s, d in runner.zero_shapes
        ]

    times = []
    for it in range(iters + 1):
        z1 = dev_zeros()
        z2 = dev_zeros()
        jax.block_until_ready(z1)
        jax.block_until_ready(z2)
        t0 = time.perf_counter()
        o1 = runner.fn(*din1, *z1)
        jax.block_until_ready(o1)
        o2 = runner.fn(*din2, *z2)
        jax.block_until_ready(o2)
        t1 = time.perf_counter()
        if it > 0:  # skip warmup
            times.append(t1 - t0)
    return times


# revision 10
# speedup vs baseline: 45.2714x; 1.0715x over previous
"""NeuralFP GNN message-passing kernel for 8 Trainium2 NeuronCores.

Strategy (graph-level data parallel, per sharding hint):
  - Nodes are partitioned into 8 contiguous ranges of 6250; each core owns
    the aggregation + MLP + softmax + graph pooling for its node range.
  - Incident edges are bucketed on host by (owner core, 128-node dst tile,
    src half-block) and gathered on device with gpsimd dma_gather (int16
    indices limit 32767 -> two 25000-row source blocks). Gathers are
    issued per PAIR of node tiles to amortize the ~1us SWDGE fixed cost.
  - segment_sum over edges: per 128-edge block, a 0/1 "onehot" matrix
    (built on DVE via is_equal against an iota table) reduces gathered
    rows into the 128 dst slots through a PE matmul accumulated in PSUM.
  - '+ h' self-loop term is folded in as explicit (v, v) edges.
  - softmax: logits are bounded (|logit| < ~8), so exp without max-shift;
    row sums come free via the ACT accumulate port; the 1/sum scaling is
    folded into the pooling matmul's onehot weights.
  - pooling: batch indices are sorted, so node tiles are grouped into <=128
    graph windows; pooling matmuls accumulate a [128, 2048] PSUM window
    across tiles, flushed per window; host overlap-adds windows/cores.
  - Two launches of the SAME compiled program (layer 1 with x/H1/W1,
    layer 2 with h1/H2/W2); host all-gathers h1 between launches and sums
    the pooled fingerprints of both layers.
"""

import os
import numpy as np

N = 50000
F = 64
FP = 2048
NCORES = 8
NPC = N // NCORES  # 6250 nodes per core
TN = 128  # node tile
NT = (NPC + TN - 1) // TN  # 49 node tiles per core
NPAIR = (NT + 1) // 2  # 25 tile pairs (last pair has 1 tile)
BLK = 25000  # src index block (int16 limit)
FPC = 512  # fp chunk (1 PSUM bank)
NCH = FP // FPC  # 4 chunks

_CACHE = {}
LAST_PROFILE = {}


def _roundup(x, m):
    return ((x + m - 1) // m) * m


def _pair_tiles(p):
    return [t for t in (2 * p, 2 * p + 1) if t < NT]


def _preprocess(edge_index, batch):
    """Bucket edges and build all per-core device tables."""
    src = np.asarray(edge_index[0], dtype=np.int64)
    dst = np.asarray(edge_index[1], dtype=np.int64)
    loop = np.arange(N, dtype=np.int64)  # self loops implement '+ h'
    src = np.concatenate([src, loop])
    dst = np.concatenate([dst, loop])
    batch = np.asarray(batch, dtype=np.int64)

    core = dst // NPC
    dst_local = dst - core * NPC
    t = dst_local // TN
    b = src // BLK
    d_in_tile = dst_local % TN
    idx_local = (src - b * BLK).astype(np.int64)

    # bucket edge lists per (core, t, b); sort by src idx within each bucket
    # (HBM page locality for the gather descriptors)
    key = ((core * NT + t) * 2 + b).astype(np.int64)
    order = np.lexsort((idx_local, key))
    key_s = key[order]
    idx_s = idx_local[order]
    dit_s = d_in_tile[order]
    counts = np.bincount(key_s, minlength=NCORES * NT * 2).reshape(NCORES, NT, 2)
    starts = np.zeros(NCORES * NT * 2 + 1, np.int64)
    np.cumsum(counts.reshape(-1), out=starts[1:])

    caps = np.maximum(
        _roundup(counts.max(axis=0), TN), TN
    )  # [NT, 2] compile-time bucket capacities

    # per-pair layout: segments in order [t0b0, t1b0, t0b1, t1b1]
    # seg_col[t][b] = column offset of that bucket inside the global dstl /
    # gathered-column space; pair_col_off[p] = start of pair p's columns.
    seg_col = np.zeros((NT, 2), np.int64)
    pair_cols = np.zeros(NPAIR, np.int64)
    pair_col_off = np.zeros(NPAIR, np.int64)
    col = 0
    for p in range(NPAIR):
        pair_col_off[p] = col
        for bb in range(2):
            for tt in _pair_tiles(p):
                seg_col[tt, bb] = col
                col += int(caps[tt, bb]) // TN
        pair_cols[p] = col - pair_col_off[p]
    total_cols = col

    # idx table offsets (16 idx per col): same order as seg_col
    ioff = np.zeros((NT, 2), np.int64)
    io = 0
    for p in range(NPAIR):
        for bb in range(2):
            for tt in _pair_tiles(p):
                ioff[tt, bb] = io
                io += int(caps[tt, bb]) // 16
    total_icols = io

    eidx = np.zeros((NCORES, 128, total_icols), np.int16)
    dstl = -np.ones((NCORES, 128, total_cols), np.float32)

    for c in range(NCORES):
        for ti in range(NT):
            for bb in range(2):
                k = (c * NT + ti) * 2 + bb
                s, e = starts[k], starts[k] + counts[c, ti, bb]
                cap = int(caps[ti, bb])
                vals = np.zeros(cap, np.int16)
                vals[: e - s] = idx_s[s:e]
                seg = vals.reshape(cap // 16, 16).T  # [16, cols]
                io = int(ioff[ti, bb])
                eidx[c, :, io : io + cap // 16] = np.tile(seg, (8, 1))
                dv = -np.ones(cap, np.float32)
                dv[: e - s] = dit_s[s:e]
                col0 = int(seg_col[ti, bb])
                dstl[c, :, col0 : col0 + cap // TN] = dv.reshape(cap // TN, TN).T

    windows = _common_windows(batch)
    NW = len(windows)
    gloc = -np.ones((NCORES, 128, NT), np.float32)
    gbases = np.zeros((NCORES, NW), np.int64)
    for c in range(NCORES):
        ng = batch[c * NPC : (c + 1) * NPC]
        for w, tl in enumerate(windows):
            gb = int(ng[tl[0] * TN])
            gbases[c, w] = gb
            for ti in tl:
                lo = ti * TN
                hi = min((ti + 1) * TN, NPC)
                span = ng[lo:hi] - gb
                assert span.max() < 128 and span.min() >= 0
                gloc[c, : hi - lo, ti] = span

    return dict(
        caps=caps,
        seg_col=seg_col,
        pair_col_off=pair_col_off,
        pair_cols=pair_cols,
        ioff=ioff,
        total_cols=total_cols,
        total_icols=total_icols,
        eidx=eidx,
        dstl=dstl,
        gloc=gloc,
        gbases=gbases,
        windows=windows,
        NW=NW,
    )


def _common_windows(batch):
    """Split the NT node tiles into windows (aligned to tile PAIRS) with
    graph span <= 128 on EVERY core simultaneously."""
    batch = np.asarray(batch, dtype=np.int64)
    spans = np.zeros((NCORES, NT, 2), np.int64)
    for c in range(NCORES):
        ng = batch[c * NPC : (c + 1) * NPC]
        for ti in range(NT):
            lo = ti * TN
            hi = min((ti + 1) * TN, NPC)
            spans[c, ti] = (ng[lo], ng[hi - 1])
    wins = []
    cur = [0, 1]
    for p in range(1, NPAIR):
        tl = _pair_tiles(p)
        ok = all(
            spans[c, tl[-1], 1] - spans[c, cur[0], 0] < 128 for c in range(NCORES)
        )
        if ok:
            cur.extend(tl)
        else:
            wins.append(cur)
            cur = list(tl)
    wins.append(cur)
    return wins


def _build_program(pp, reps=1):
    import concourse.bacc as bacc
    import concourse.mybir as mybir
    import concourse.tile as tile

    caps = pp["caps"]
    seg_col = pp["seg_col"]
    ioff = pp["ioff"]
    total_cols = pp["total_cols"]
    total_icols = pp["total_icols"]
    windows = pp["windows"]
    NW = pp["NW"]
    pair_cols = pp["pair_cols"]
    pair_col_off = pp["pair_col_off"]
    CT_MAX = int(pair_cols.max())

    nc = bacc.Bacc("TRN2", target_bir_lowering=False, debug=False, num_swdge_queues=4)
    f32 = mybir.dt.float32
    h = nc.dram_tensor("h", [N, F], f32, kind="ExternalInput")
    hwt = nc.dram_tensor("hwt", [F, F], f32, kind="ExternalInput")
    hwt_ext = nc.dram_tensor("hwt_ext", [F + 1, F], f32, kind="ExternalInput")
    hb = nc.dram_tensor("hb", [F, 1], f32, kind="ExternalInput")
    wwt_ext = nc.dram_tensor("wwt_ext", [F + 1, FP], f32, kind="ExternalInput")
    iota = nc.dram_tensor("iota", [128, 128], f32, kind="ExternalInput")
    eidx = nc.dram_tensor("eidx", [128, total_icols], mybir.dt.int16, kind="ExternalInput")
    dstl = nc.dram_tensor("dstl", [128, total_cols], f32, kind="ExternalInput")
    gloc = nc.dram_tensor("gloc", [128, NT], f32, kind="ExternalInput")
    h_out = nc.dram_tensor("h_out", [NPC, F], f32, kind="ExternalOutput")
    pooled = nc.dram_tensor("pooled", [NW, 128, FP], f32, kind="ExternalOutput")

    with tile.TileContext(nc) as tc:
        with (
            tc.tile_pool(name="const", bufs=1) as cpool,
            tc.tile_pool(name="gath", bufs=2) as gpool,
            tc.tile_pool(name="oh", bufs=4) as ohpool,
            tc.tile_pool(name="sb", bufs=3) as sbpool,
            tc.tile_pool(name="exp", bufs=2) as epool,
            tc.tile_pool(name="pw", bufs=2) as pwpool,
            tc.tile_pool(name="ps_small", bufs=3, space="PSUM") as pss,
            tc.tile_pool(name="ps_fp", bufs=1, space="PSUM") as psf,
            tc.tile_pool(name="ps_win", bufs=1, space="PSUM") as psw,
        ):
            eidx_sb = cpool.tile([128, total_icols], mybir.dt.int16)
            nc.sync.dma_start(out=eidx_sb[:], in_=eidx[:])
            dstl_sb = cpool.tile([128, total_cols], f32)
            nc.sync.dma_start(out=dstl_sb[:], in_=dstl[:])
            gloc_sb = cpool.tile([128, NT], f32)
            nc.sync.dma_start(out=gloc_sb[:], in_=gloc[:])
            iota_sb = cpool.tile([128, 128], f32)
            nc.sync.dma_start(out=iota_sb[:], in_=iota[:])
            hwt_sb = cpool.tile([F, F], f32)
            nc.sync.dma_start(out=hwt_sb[:], in_=hwt[:])
            hwt_ext_sb = cpool.tile([F + 1, F], f32)
            nc.sync.dma_start(out=hwt_ext_sb[:], in_=hwt_ext[:])
            hb_sb = cpool.tile([F, 1], f32)
            nc.sync.dma_start(out=hb_sb[:], in_=hb[:])
            wwt_sb = cpool.tile([F + 1, FP], f32)
            nc.sync.dma_start(out=wwt_sb[:], in_=wwt_ext[:])

            gath_tiles = {}  # pair -> (tile handle, base col)
            _rep = 0
            qctr = [0]

            def ensure_gather(p):
                if p in gath_tiles:
                    return gath_tiles[p]
                base = int(pair_col_off[p])
                gt = gpool.tile([128, CT_MAX, F], f32, tag="gath")
                # two calls: block 0 covers [t0b0, t1b0], block 1 [t0b1, t1b1]
                tl = _pair_tiles(p)
                for bb in range(2):
                    cap = sum(int(caps[tt, bb]) for tt in tl)
                    io = int(ioff[tl[0], bb])
                    colo = int(seg_col[tl[0], bb]) - base
                    nc.gpsimd.dma_gather(
                        gt[:, colo : colo + cap // TN, :],
                        h[bb * BLK : (bb + 1) * BLK, :],
                        eidx_sb[:, io : io + cap // 16],
                        cap,
                        cap,
                        F,
                        elem_step=F,
                        single_packet=False,
                    )
                gath_tiles.clear()
                gath_tiles[p] = (gt, base)
                return gath_tiles[p]

            for _rep, (w, tl_w) in [
                (r, wt) for r in range(reps) for wt in enumerate(windows)
            ]:
                psum_win = psw.tile([128, FP], f32)
                for wi, t in enumerate(tl_w):
                    gt, gbase = ensure_gather(t // 2)
                    # block columns of this tile inside the pair tile
                    bcols = []
                    for bb in range(2):
                        c0 = int(seg_col[t, bb]) - gbase
                        bcols.extend(range(c0, c0 + int(caps[t, bb]) // TN))
                    psum_aggT = pss.tile([F, 128], f32, tag="pss")
                    for bi, blk in enumerate(bcols):
                        oh = ohpool.tile([128, 128], f32, tag="oh")
                        dc = gbase + blk
                        nc.vector.tensor_tensor(
                            out=oh[:],
                            in0=dstl_sb[:, dc : dc + 1].to_broadcast([128, 128]),
                            in1=iota_sb[:],
                            op=mybir.AluOpType.is_equal,
                        )
                        nc.tensor.matmul(
                            psum_aggT[:],
                            lhsT=gt[:, blk, :],
                            rhs=oh[:],
                            start=(bi == 0),
                            stop=(bi == len(bcols) - 1),
                        )
                    aggT_ext = sbpool.tile([F + 1, 128], f32, tag="aggT")
                    nc.vector.memset(aggT_ext[F : F + 1, :], 1.0)
                    nc.scalar.copy(aggT_ext[0:F, :], psum_aggT[:])
                    # updT = sigmoid(Hw @ aggT + hb)
                    psum_updT = pss.tile([F, 128], f32, tag="pss")
                    nc.tensor.matmul(
                        psum_updT[:], lhsT=hwt_sb[:], rhs=aggT_ext[0:F, :],
                        start=True, stop=True,
                    )
                    updT_ext = sbpool.tile([F + 1, 128], f32, tag="updT")
                    nc.vector.memset(updT_ext[F : F + 1, :], 1.0)
                    nc.scalar.activation(
                        updT_ext[0:F, :],
                        psum_updT[:],
                        mybir.ActivationFunctionType.Sigmoid,
                        bias=hb_sb[:],
                    )
                    # upd (row-major) = sigmoid(agg @ Hw.T + Hb) for h_out
                    psum_upd = pss.tile([128, F], f32, tag="pss")
                    nc.tensor.matmul(
                        psum_upd[:], lhsT=aggT_ext[:], rhs=hwt_ext_sb[:],
                        start=True, stop=True,
                    )
                    upd_sb = sbpool.tile([128, F], f32, tag="upd")
                    nc.scalar.activation(
                        upd_sb[:], psum_upd[:], mybir.ActivationFunctionType.Sigmoid
                    )
                    rows = min(TN, NPC - t * TN)
                    nc.sync.dma_start(
                        out=h_out[t * TN : t * TN + rows, :], in_=upd_sb[0:rows, :]
                    )
                    # logits -> exp (+row sums) per 512 chunk
                    exp_sb = epool.tile([128, FP], f32, tag="exp")
                    rs4 = sbpool.tile([128, NCH], f32, tag="rs4")
                    for ch in range(NCH):
                        psum_fp = psf.tile([128, FPC], f32, tag="fp")
                        nc.tensor.matmul(
                            psum_fp[:],
                            lhsT=updT_ext[:],
                            rhs=wwt_sb[:, ch * FPC : (ch + 1) * FPC],
                            start=True,
                            stop=True,
                        )
                        nc.scalar.activation(
                            exp_sb[:, ch * FPC : (ch + 1) * FPC],
                            psum_fp[:],
                            mybir.ActivationFunctionType.Exp,
                            accum_out=rs4[:, ch : ch + 1],
                        )
                    rs = sbpool.tile([128, 1], f32, tag="rs")
                    nc.vector.tensor_reduce(
                        rs[:], rs4[:], mybir.AxisListType.X, mybir.AluOpType.add
                    )
                    recip = sbpool.tile([128, 1], f32, tag="recip")
                    nc.vector.reciprocal(recip[:], rs[:])
                    woh = ohpool.tile([128, 128], f32, tag="woh")
                    nc.vector.tensor_tensor(
                        out=woh[:],
                        in0=gloc_sb[:, t : t + 1].to_broadcast([128, 128]),
                        in1=iota_sb[:],
                        op=mybir.AluOpType.is_equal,
                    )
                    nc.vector.tensor_scalar(
                        out=woh[:],
                        in0=woh[:],
                        scalar1=recip[:],
                        scalar2=None,
                        op0=mybir.AluOpType.mult,
                    )
                    for ch in range(NCH):
                        nc.tensor.matmul(
                            psum_win[:, ch * FPC : (ch + 1) * FPC],
                            lhsT=woh[:],
                            rhs=exp_sb[:, ch * FPC : (ch + 1) * FPC],
                            start=(wi == 0),
                            stop=(wi == len(tl_w) - 1),
                        )
                pw_sb = pwpool.tile([128, FP], f32, tag="pw")
                nc.scalar.copy(pw_sb[:], psum_win[:])
                nc.sync.dma_start(out=pooled[w], in_=pw_sb[:])

    nc.compile()
    return nc


class _Runner:
    """Cached shard_map executor for one compiled Bass program (replicates
    bass2jax.run_bass_via_pjrt but builds the jitted callable once)."""

    def __init__(self, nc, n_cores):
        import jax
        import concourse.mybir as mybir
        from concourse import bass2jax
        from jax.sharding import Mesh, PartitionSpec
        from jax.experimental.shard_map import shard_map

        bass2jax.install_neuronx_cc_hook()
        self.nc = nc
        self.n_cores = n_cores
        partition_name = (
            nc.partition_id_tensor.name if nc.partition_id_tensor else None
        )
        in_names, out_names, out_avals, zero_shapes = [], [], [], []
        for alloc in nc.m.functions[0].allocations:
            if not isinstance(alloc, mybir.MemoryLocationSet):
                continue
            name = alloc.memorylocations[0].name
            if alloc.kind == "ExternalInput":
                if name != partition_name:
                    in_names.append(name)
            elif alloc.kind == "ExternalOutput":
                shape = tuple(alloc.tensor_shape)
                dtype = mybir.dt.np(alloc.dtype)
                out_names.append(name)
                out_avals.append(jax.core.ShapedArray(shape, dtype))
                zero_shapes.append((shape, dtype))
        self.in_names = in_names
        self.out_names = out_names
        self.zero_shapes = zero_shapes
        self.out_avals = out_avals
        n_params = len(in_names)
        n_outs = len(out_avals)
        all_in_names = list(in_names) + list(out_names)
        if partition_name is not None:
            all_in_names.append(partition_name)

        def _body(*args):
            operands = list(args)
            if partition_name is not None:
                operands.append(bass2jax.partition_id_tensor())
            outs = bass2jax._bass_exec_p.bind(
                *operands,
                out_avals=tuple(out_avals),
                in_names=tuple(all_in_names),
                out_names=tuple(out_names),
                lowering_input_output_aliases=(),
                sim_require_finite=True,
                sim_require_nnan=True,
                nc=nc,
            )
            return tuple(outs)

        devices = jax.devices()[:n_cores]
        assert len(devices) == n_cores
        mesh = Mesh(np.asarray(devices), ("core",))
        in_specs = (PartitionSpec("core"),) * (n_params + n_outs)
        out_specs = (PartitionSpec("core"),) * n_outs
        donate = tuple(range(n_params, n_params + n_outs))
        self.fn = jax.jit(
            shard_map(
                _body, mesh=mesh, in_specs=in_specs, out_specs=out_specs,
                check_rep=False,
            ),
            donate_argnums=donate,
            keep_unused=True,
        )

    def run_async(self, in_maps):
        concat_in = [
            np.concatenate([np.asarray(m[name]) for m in in_maps], axis=0)
            for name in self.in_names
        ]
        zeros = [
            np.zeros((self.n_cores * s[0], *s[1:]), d) for s, d in self.zero_shapes
        ]
        return self.fn(*concat_in, *zeros)

    def __call__(self, in_maps):
        out_arrs = self.run_async(in_maps)
        nc_ = self.n_cores
        return [
            {
                name: np.asarray(out_arrs[i]).reshape(
                    nc_, *self.out_avals[i].shape
                )[c]
                for i, name in enumerate(self.out_names)
            }
            for c in range(nc_)
        ]


def _get_compiled(edge_index, batch):
    import hashlib

    hsh = hashlib.sha256()
    hsh.update(np.ascontiguousarray(edge_index).tobytes())
    hsh.update(np.ascontiguousarray(batch).tobytes())
    key = hsh.hexdigest()
    if key not in _CACHE:
        pp = _preprocess(edge_index, batch)
        nc = _build_program(pp)
        runner = _Runner(nc, NCORES)
        _CACHE.clear()
        _CACHE[key] = (pp, runner)
    return _CACHE[key]


def _weights_maps(pp, h_full, Hw, Hb, Ww, Wb):
    hwt = np.ascontiguousarray(Hw.T.astype(np.float32))
    hwt_ext = np.vstack([hwt, Hb.astype(np.float32)[None, :]])
    hbcol = np.ascontiguousarray(Hb.astype(np.float32)[:, None])
    wwt_ext = np.vstack(
        [np.ascontiguousarray(Ww.T.astype(np.float32)), Wb.astype(np.float32)[None, :]]
    )
    iota_np = np.tile(np.arange(128, dtype=np.float32), (128, 1))
    return [
        {
            "h": h_full,
            "hwt": hwt,
            "hwt_ext": hwt_ext,
            "hb": hbcol,
            "wwt_ext": wwt_ext,
            "iota": iota_np,
            "eidx": pp["eidx"][c],
            "dstl": pp["dstl"][c],
            "gloc": pp["gloc"][c],
        }
        for c in range(NCORES)
    ]


def kernel(
    x,
    H1_w,
    H1_b,
    W1_w,
    W1_b,
    H2_w,
    H2_b,
    W2_w,
    W2_b,
    edge_index,
    batch,
    num_graphs,
):
    x = np.asarray(x, dtype=np.float32)
    num_graphs = int(num_graphs)
    pp, runner = _get_compiled(edge_index, batch)
    NW = pp["NW"]

    args1 = [np.asarray(a, np.float32) for a in (H1_w, H1_b, W1_w, W1_b)]
    args2 = [np.asarray(a, np.float32) for a in (H2_w, H2_b, W2_w, W2_b)]

    res1 = runner(_weights_maps(pp, x, *args1))
    h1 = np.ascontiguousarray(
        np.concatenate([res1[c]["h_out"] for c in range(NCORES)], axis=0)
    )
    res2 = runner(_weights_maps(pp, h1, *args2))

    out = np.zeros((num_graphs, FP), np.float32)
    for res in (res1, res2):
        for c in range(NCORES):
            for w in range(NW):
                gb = int(pp["gbases"][c, w])
                rows = min(128, num_graphs - gb)
                out[gb : gb + rows] += res[c]["pooled"][w][:rows]
    return out


def benchmark(inputs, iters=5):
    """Time the two on-device launches with inputs pre-placed on device
    (excludes host preprocessing + h2d; donated output zeros are created
    on-device). Returns per-iteration seconds for launch1+launch2."""
    import time
    import jax
    import jax.numpy as jnp
    from jax.sharding import NamedSharding, PartitionSpec

    x = np.asarray(inputs["x"], np.float32)
    pp, runner = _get_compiled(inputs["edge_index"], inputs["batch"])
    args1 = [np.asarray(inputs[k], np.float32) for k in ("H1_w", "H1_b", "W1_w", "W1_b")]
    args2 = [np.asarray(inputs[k], np.float32) for k in ("H2_w", "H2_b", "W2_w", "W2_b")]

    res1 = runner(_weights_maps(pp, x, *args1))
    h1 = np.ascontiguousarray(
        np.concatenate([res1[c]["h_out"] for c in range(NCORES)], axis=0)
    )
    maps1 = _weights_maps(pp, x, *args1)
    maps2 = _weights_maps(pp, h1, *args2)

    mesh = runner.fn.__wrapped__ if False else None  # noqa
    devices = jax.devices()[:NCORES]
    from jax.sharding import Mesh

    mesh = Mesh(np.asarray(devices), ("core",))
    sh = NamedSharding(mesh, PartitionSpec("core"))

    def dev_inputs(maps):
        return [
            jax.device_put(
                np.concatenate([np.asarray(m[name]) for m in maps], axis=0), sh
            )
            for name in runner.in_names
        ]

    din1 = dev_inputs(maps1)
    din2 = dev_inputs(maps2)

    def dev_zeros():
        return [
            jax.device_put(jnp.zeros((NCORES * s[0], *s[1:]), d), sh)
            for s, d in runner.zero_shapes
        ]

    times = []
    for it in range(iters + 1):
        z1 = dev_zeros()
        z2 = dev_zeros()
        jax.block_until_ready(z1)
        jax.block_until_ready(z2)
        t0 = time.perf_counter()
        o1 = runner.fn(*din1, *z1)
        jax.block_until_ready(o1)
        o2 = runner.fn(*din2, *z2)
        jax.block_until_ready(o2)
        t1 = time.perf_counter()
        if it > 0:  # skip warmup
            times.append(t1 - t0)
    return times


# revision 13
# speedup vs baseline: 93.4029x; 2.0632x over previous
"""NeuralFP GNN message-passing kernel for 8 Trainium2 NeuronCores.

Strategy (graph-level data parallel, per sharding hint):
  - Nodes are partitioned into 8 contiguous ranges of 6250; each core owns
    the aggregation + MLP + softmax + graph pooling for its node range.
  - Incident edges are bucketed on host by (owner core, 128-node dst tile,
    src half-block) and gathered on device with gpsimd dma_gather (int16
    indices limit 32767 -> two 25000-row source blocks), spread over 4
    SWDGE queues for descriptor-level parallelism.
  - segment_sum over edges: per 128-edge block, a 0/1 "onehot" matrix
    (built on DVE via is_equal against an iota table) reduces gathered
    rows into the 128 dst slots through a PE matmul accumulated in PSUM.
  - '+ h' self-loop term is folded in as explicit (v, v) edges.
  - softmax: logits are bounded (|logit| < ~8), so exp without max-shift;
    row sums come free via the ACT accumulate port; the 1/sum scaling is
    folded into the pooling matmul's onehot weights.
  - pooling: batch indices are sorted, so node tiles are grouped into <=128
    graph windows; pooling matmuls accumulate a [128, 2048] PSUM window
    across tiles; layer-1 windows park in SBUF, layer-2 adds and flushes.
  - BOTH layers run in ONE launch: layer 1 writes its h1 slice to internal
    DRAM, an on-device AllGather replicates h1 across the 8 cores, layer 2
    gathers from the replicated copy. Host overlap-adds the per-core
    per-window pooled outputs into the [num_graphs, 2048] result.
"""

import os
import numpy as np

N = 50000
F = 64
FP = 2048
NCORES = 8
NPC = N // NCORES  # 6250 nodes per core
TN = 128  # node tile
NT = (NPC + TN - 1) // TN  # 49 node tiles per core
NPAIR = (NT + 1) // 2  # 25 tile pairs (last pair has 1 tile)
BLK = 25000  # src index block (int16 limit)
FPC = 512  # fp chunk (1 PSUM bank)
NCH = FP // FPC  # 4 chunks

_CACHE = {}


def _roundup(x, m):
    return ((x + m - 1) // m) * m


def _pair_tiles(p):
    return [t for t in (2 * p, 2 * p + 1) if t < NT]


def _preprocess(edge_index, batch):
    """Bucket edges and build all per-core device tables."""
    src = np.asarray(edge_index[0], dtype=np.int64)
    dst = np.asarray(edge_index[1], dtype=np.int64)
    loop = np.arange(N, dtype=np.int64)  # self loops implement '+ h'
    src = np.concatenate([src, loop])
    dst = np.concatenate([dst, loop])
    batch = np.asarray(batch, dtype=np.int64)

    core = dst // NPC
    dst_local = dst - core * NPC
    t = dst_local // TN
    b = src // BLK
    d_in_tile = dst_local % TN
    idx_local = (src - b * BLK).astype(np.int64)

    # bucket edge lists per (core, t, b); sort by src idx within each bucket
    # (HBM page locality for the gather descriptors)
    key = ((core * NT + t) * 2 + b).astype(np.int64)
    order = np.lexsort((idx_local, key))
    key_s = key[order]
    idx_s = idx_local[order]
    dit_s = d_in_tile[order]
    counts = np.bincount(key_s, minlength=NCORES * NT * 2).reshape(NCORES, NT, 2)
    starts = np.zeros(NCORES * NT * 2 + 1, np.int64)
    np.cumsum(counts.reshape(-1), out=starts[1:])

    caps = np.maximum(
        _roundup(counts.max(axis=0), TN), TN
    )  # [NT, 2] compile-time bucket capacities

    # per-pair layout: segments in order [t0b0, t1b0, t0b1, t1b1]
    seg_col = np.zeros((NT, 2), np.int64)
    pair_cols = np.zeros(NPAIR, np.int64)
    pair_col_off = np.zeros(NPAIR, np.int64)
    col = 0
    for p in range(NPAIR):
        pair_col_off[p] = col
        for bb in range(2):
            for tt in _pair_tiles(p):
                seg_col[tt, bb] = col
                col += int(caps[tt, bb]) // TN
        pair_cols[p] = col - pair_col_off[p]
    total_cols = col

    ioff = np.zeros((NT, 2), np.int64)
    io = 0
    for p in range(NPAIR):
        for bb in range(2):
            for tt in _pair_tiles(p):
                ioff[tt, bb] = io
                io += int(caps[tt, bb]) // 16
    total_icols = io

    eidx = np.zeros((NCORES, 128, total_icols), np.int16)
    dstl = -np.ones((NCORES, 128, total_cols), np.float32)

    for c in range(NCORES):
        for ti in range(NT):
            for bb in range(2):
                k = (c * NT + ti) * 2 + bb
                s, e = starts[k], starts[k] + counts[c, ti, bb]
                cap = int(caps[ti, bb])
                vals = np.zeros(cap, np.int16)
                vals[: e - s] = idx_s[s:e]
                seg = vals.reshape(cap // 16, 16).T  # [16, cols]
                io = int(ioff[ti, bb])
                eidx[c, :, io : io + cap // 16] = np.tile(seg, (8, 1))
                dv = -np.ones(cap, np.float32)
                dv[: e - s] = dit_s[s:e]
                col0 = int(seg_col[ti, bb])
                dstl[c, :, col0 : col0 + cap // TN] = dv.reshape(cap // TN, TN).T

    windows = _common_windows(batch)
    NW = len(windows)
    gloc = -np.ones((NCORES, 128, NT), np.float32)
    gbases = np.zeros((NCORES, NW), np.int64)
    for c in range(NCORES):
        ng = batch[c * NPC : (c + 1) * NPC]
        for w, tl in enumerate(windows):
            gb = int(ng[tl[0] * TN])
            gbases[c, w] = gb
            for ti in tl:
                lo = ti * TN
                hi = min((ti + 1) * TN, NPC)
                span = ng[lo:hi] - gb
                assert span.max() < 128 and span.min() >= 0
                gloc[c, : hi - lo, ti] = span

    return dict(
        caps=caps,
        seg_col=seg_col,
        pair_col_off=pair_col_off,
        pair_cols=pair_cols,
        ioff=ioff,
        total_cols=total_cols,
        total_icols=total_icols,
        eidx=eidx,
        dstl=dstl,
        gloc=gloc,
        gbases=gbases,
        windows=windows,
        NW=NW,
    )


def _common_windows(batch):
    """Split the NT node tiles into windows (aligned to tile PAIRS) with
    graph span <= 128 on EVERY core simultaneously."""
    batch = np.asarray(batch, dtype=np.int64)
    spans = np.zeros((NCORES, NT, 2), np.int64)
    for c in range(NCORES):
        ng = batch[c * NPC : (c + 1) * NPC]
        for ti in range(NT):
            lo = ti * TN
            hi = min((ti + 1) * TN, NPC)
            spans[c, ti] = (ng[lo], ng[hi - 1])
    wins = []
    cur = [0, 1]
    for p in range(1, NPAIR):
        tl = _pair_tiles(p)
        ok = all(
            spans[c, tl[-1], 1] - spans[c, cur[0], 0] < 128 for c in range(NCORES)
        )
        if ok:
            cur.extend(tl)
        else:
            wins.append(cur)
            cur = list(tl)
    wins.append(cur)
    return wins


def _build_program(pp, reps=1):
    import concourse.bacc as bacc
    import concourse.mybir as mybir
    import concourse.tile as tile

    caps = pp["caps"]
    seg_col = pp["seg_col"]
    ioff = pp["ioff"]
    total_cols = pp["total_cols"]
    total_icols = pp["total_icols"]
    windows = pp["windows"]
    NW = pp["NW"]
    pair_cols = pp["pair_cols"]
    pair_col_off = pp["pair_col_off"]
    CT_MAX = int(pair_cols.max())

    nc = bacc.Bacc(
        "TRN2", target_bir_lowering=False, debug=False, num_swdge_queues=4
    )
    f32 = mybir.dt.float32
    h = nc.dram_tensor("h", [N, F], f32, kind="ExternalInput")
    wts = []
    for L in (1, 2):
        wts.append(
            dict(
                hwt=nc.dram_tensor(f"hwt{L}", [F, F], f32, kind="ExternalInput"),
                hwt_ext=nc.dram_tensor(
                    f"hwt_ext{L}", [F + 1, F], f32, kind="ExternalInput"
                ),
                hb=nc.dram_tensor(f"hb{L}", [F, 1], f32, kind="ExternalInput"),
                wwt_ext=nc.dram_tensor(
                    f"wwt_ext{L}", [F + 1, FP], f32, kind="ExternalInput"
                ),
            )
        )
    iota = nc.dram_tensor("iota", [128, 128], f32, kind="ExternalInput")
    eidx = nc.dram_tensor(
        "eidx", [128, total_icols], mybir.dt.int16, kind="ExternalInput"
    )
    dstl = nc.dram_tensor("dstl", [128, total_cols], f32, kind="ExternalInput")
    gloc = nc.dram_tensor("gloc", [128, NT], f32, kind="ExternalInput")
    pooled = nc.dram_tensor("pooled", [NW, 128, FP], f32, kind="ExternalOutput")
    h1_mine = nc.dram_tensor("h1_mine", [NPC, F], f32)
    h1_all = nc.dram_tensor("h1_all", [N, F], f32, addr_space="Shared")

    with tile.TileContext(nc) as tc:
        with (
            tc.tile_pool(name="const", bufs=1) as cpool,
            tc.tile_pool(name="gath", bufs=4) as gpool,
            tc.tile_pool(name="oh", bufs=8) as ohpool,
            tc.tile_pool(name="sb", bufs=4) as sbpool,
            tc.tile_pool(name="exp", bufs=3) as epool,
            tc.tile_pool(name="pw", bufs=2) as pwpool,
            tc.tile_pool(name="ps_small", bufs=3, space="PSUM") as pss,
            tc.tile_pool(name="ps_fp", bufs=1, space="PSUM") as psf,
            tc.tile_pool(name="ps_win", bufs=1, space="PSUM") as psw,
        ):
            eidx_sb = cpool.tile([128, total_icols], mybir.dt.int16)
            nc.sync.dma_start(out=eidx_sb[:], in_=eidx[:])
            dstl_sb = cpool.tile([128, total_cols], f32)
            nc.sync.dma_start(out=dstl_sb[:], in_=dstl[:])
            gloc_sb = cpool.tile([128, NT], f32)
            nc.sync.dma_start(out=gloc_sb[:], in_=gloc[:])
            iota_sb = cpool.tile([128, 128], f32)
            nc.sync.dma_start(out=iota_sb[:], in_=iota[:])
            wsb = []
            for L in (0, 1):
                d = {}
                d["hwt"] = cpool.tile([F, F], f32, tag=f"hwt{L}", name=f"hwt{L}_sb")
                nc.sync.dma_start(out=d["hwt"][:], in_=wts[L]["hwt"][:])
                d["hwt_ext"] = cpool.tile([F + 1, F], f32, tag=f"hwte{L}", name=f"hwte{L}_sb")
                nc.sync.dma_start(out=d["hwt_ext"][:], in_=wts[L]["hwt_ext"][:])
                d["hb"] = cpool.tile([F, 1], f32, tag=f"hb{L}", name=f"hb{L}_sb")
                nc.sync.dma_start(out=d["hb"][:], in_=wts[L]["hb"][:])
                d["wwt"] = cpool.tile([F + 1, FP], f32, tag=f"wwt{L}", name=f"wwt{L}_sb")
                nc.sync.dma_start(out=d["wwt"][:], in_=wts[L]["wwt_ext"][:])
                wsb.append(d)
            pw_acc = [
                cpool.tile([128, FP], f32, tag=f"pwacc{w}", name=f"pwacc{w}_sb") for w in range(NW)
            ]

            qctr = [0]
            gath_tiles = {}

            def emit_gathers(p, src_dram):
                if p in gath_tiles:
                    return gath_tiles[p]
                base = int(pair_col_off[p])
                gt = gpool.tile([128, CT_MAX, F], f32, tag="gath")
                tl = _pair_tiles(p)
                for bb in range(2):
                    for tt in tl:
                        cap = int(caps[tt, bb])
                        io = int(ioff[tt, bb])
                        colo = int(seg_col[tt, bb]) - base
                        nc.gpsimd.dma_gather(
                            gt[:, colo : colo + cap // TN, :],
                            src_dram[bb * BLK : (bb + 1) * BLK, :],
                            eidx_sb[:, io : io + cap // 16],
                            cap,
                            cap,
                            F,
                            elem_step=F,
                            single_packet=False,
                            queue_num=qctr[0] % 4,
                        )
                        qctr[0] += 1
                gath_tiles.clear()
                gath_tiles[p] = (gt, base)
                return gath_tiles[p]

            for _rep in range(reps):
                for L in (0, 1):
                    W = wsb[L]
                    src_dram = h if L == 0 else h1_all
                    gath_tiles.clear()
                    for w, tl_w in enumerate(windows):
                        psum_win = psw.tile([128, FP], f32)
                        for wi, t in enumerate(tl_w):
                            gt, gbase = emit_gathers(t // 2, src_dram)
                            bcols = []
                            for bb in range(2):
                                c0 = int(seg_col[t, bb]) - gbase
                                bcols.extend(
                                    range(c0, c0 + int(caps[t, bb]) // TN)
                                )
                            psum_aggT = pss.tile([F, 128], f32, tag="pss")
                            for bi, blk in enumerate(bcols):
                                oh = ohpool.tile([128, 128], f32, tag="oh")
                                dc = gbase + blk
                                nc.vector.tensor_tensor(
                                    out=oh[:],
                                    in0=dstl_sb[:, dc : dc + 1].to_broadcast(
                                        [128, 128]
                                    ),
                                    in1=iota_sb[:],
                                    op=mybir.AluOpType.is_equal,
                                )
                                nc.tensor.matmul(
                                    psum_aggT[:],
                                    lhsT=gt[:, blk, :],
                                    rhs=oh[:],
                                    start=(bi == 0),
                                    stop=(bi == len(bcols) - 1),
                                )
                            aggT_ext = sbpool.tile([F + 1, 128], f32, tag="aggT")
                            nc.vector.memset(aggT_ext[F : F + 1, :], 1.0)
                            nc.scalar.copy(aggT_ext[0:F, :], psum_aggT[:])
                            # updT = sigmoid(Hw @ aggT + hb)
                            psum_updT = pss.tile([F, 128], f32, tag="pss")
                            nc.tensor.matmul(
                                psum_updT[:],
                                lhsT=W["hwt"][:],
                                rhs=aggT_ext[0:F, :],
                                start=True,
                                stop=True,
                            )
                            updT_ext = sbpool.tile([F + 1, 128], f32, tag="updT")
                            nc.vector.memset(updT_ext[F : F + 1, :], 1.0)
                            nc.scalar.activation(
                                updT_ext[0:F, :],
                                psum_updT[:],
                                mybir.ActivationFunctionType.Sigmoid,
                                bias=W["hb"][:],
                            )
                            if L == 0:
                                # row-major upd for the h1 exchange
                                psum_upd = pss.tile([128, F], f32, tag="pss")
                                nc.tensor.matmul(
                                    psum_upd[:],
                                    lhsT=aggT_ext[:],
                                    rhs=W["hwt_ext"][:],
                                    start=True,
                                    stop=True,
                                )
                                upd_sb = sbpool.tile([128, F], f32, tag="upd")
                                nc.scalar.activation(
                                    upd_sb[:],
                                    psum_upd[:],
                                    mybir.ActivationFunctionType.Sigmoid,
                                )
                                rows = min(TN, NPC - t * TN)
                                nc.sync.dma_start(
                                    out=h1_mine[t * TN : t * TN + rows, :],
                                    in_=upd_sb[0:rows, :],
                                )
                            exp_sb = epool.tile([128, FP], f32, tag="exp")
                            rs4 = sbpool.tile([128, NCH], f32, tag="rs4")
                            for ch in range(NCH):
                                psum_fp = psf.tile([128, FPC], f32, tag="fp")
                                nc.tensor.matmul(
                                    psum_fp[:],
                                    lhsT=updT_ext[:],
                                    rhs=W["wwt"][:, ch * FPC : (ch + 1) * FPC],
                                    start=True,
                                    stop=True,
                                )
                                nc.scalar.activation(
                                    exp_sb[:, ch * FPC : (ch + 1) * FPC],
                                    psum_fp[:],
                                    mybir.ActivationFunctionType.Exp,
                                    accum_out=rs4[:, ch : ch + 1],
                                )
                            rs = sbpool.tile([128, 1], f32, tag="rs")
                            nc.vector.tensor_reduce(
                                rs[:],
                                rs4[:],
                                mybir.AxisListType.X,
                                mybir.AluOpType.add,
                            )
                            recip = sbpool.tile([128, 1], f32, tag="recip")
                            nc.vector.reciprocal(recip[:], rs[:])
                            woh = ohpool.tile([128, 128], f32, tag="woh")
                            nc.vector.tensor_tensor(
                                out=woh[:],
                                in0=gloc_sb[:, t : t + 1].to_broadcast([128, 128]),
                                in1=iota_sb[:],
                                op=mybir.AluOpType.is_equal,
                            )
                            nc.vector.tensor_scalar(
                                out=woh[:],
                                in0=woh[:],
                                scalar1=recip[:],
                                scalar2=None,
                                op0=mybir.AluOpType.mult,
                            )
                            for ch in range(NCH):
                                nc.tensor.matmul(
                                    psum_win[:, ch * FPC : (ch + 1) * FPC],
                                    lhsT=woh[:],
                                    rhs=exp_sb[:, ch * FPC : (ch + 1) * FPC],
                                    start=(wi == 0),
                                    stop=(wi == len(tl_w) - 1),
                                )
                        if L == 0:
                            nc.scalar.copy(pw_acc[w][:], psum_win[:])
                        else:
                            pw_sb = pwpool.tile([128, FP], f32, tag="pw")
                            nc.vector.tensor_tensor(
                                out=pw_sb[:],
                                in0=pw_acc[w][:],
                                in1=psum_win[:],
                                op=mybir.AluOpType.add,
                            )
                            nc.sync.dma_start(out=pooled[w], in_=pw_sb[:])
                    if L == 0:
                        nc.gpsimd.collective_compute(
                            "AllGather",
                            mybir.AluOpType.bypass,
                            replica_groups=[list(range(NCORES))],
                            ins=[h1_mine[:]],
                            outs=[h1_all[:]],
                        )

    nc.compile()
    return nc


class _Runner:
    """Cached shard_map executor for one compiled Bass program (replicates
    bass2jax.run_bass_via_pjrt but builds the jitted callable once)."""

    def __init__(self, nc, n_cores):
        import jax
        import concourse.mybir as mybir
        from concourse import bass2jax
        from jax.sharding import Mesh, PartitionSpec
        from jax.experimental.shard_map import shard_map

        bass2jax.install_neuronx_cc_hook()
        self.nc = nc
        self.n_cores = n_cores
        partition_name = (
            nc.partition_id_tensor.name if nc.partition_id_tensor else None
        )
        in_names, out_names, out_avals, zero_shapes = [], [], [], []
        for alloc in nc.m.functions[0].allocations:
            if not isinstance(alloc, mybir.MemoryLocationSet):
                continue
            name = alloc.memorylocations[0].name
            if alloc.kind == "ExternalInput":
                if name != partition_name:
                    in_names.append(name)
            elif alloc.kind == "ExternalOutput":
                shape = tuple(alloc.tensor_shape)
                dtype = mybir.dt.np(alloc.dtype)
                out_names.append(name)
                out_avals.append(jax.core.ShapedArray(shape, dtype))
                zero_shapes.append((shape, dtype))
        self.in_names = in_names
        self.out_names = out_names
        self.zero_shapes = zero_shapes
        self.out_avals = out_avals
        n_params = len(in_names)
        n_outs = len(out_avals)
        all_in_names = list(in_names) + list(out_names)
        if partition_name is not None:
            all_in_names.append(partition_name)

        def _body(*args):
            operands = list(args)
            if partition_name is not None:
                operands.append(bass2jax.partition_id_tensor())
            outs = bass2jax._bass_exec_p.bind(
                *operands,
                out_avals=tuple(out_avals),
                in_names=tuple(all_in_names),
                out_names=tuple(out_names),
                lowering_input_output_aliases=(),
                sim_require_finite=True,
                sim_require_nnan=True,
                nc=nc,
            )
            return tuple(outs)

        devices = jax.devices()[:n_cores]
        assert len(devices) == n_cores
        mesh = Mesh(np.asarray(devices), ("core",))
        in_specs = (PartitionSpec("core"),) * (n_params + n_outs)
        out_specs = (PartitionSpec("core"),) * n_outs
        donate = tuple(range(n_params, n_params + n_outs))
        self.fn = jax.jit(
            shard_map(
                _body, mesh=mesh, in_specs=in_specs, out_specs=out_specs,
                check_rep=False,
            ),
            donate_argnums=donate,
            keep_unused=True,
        )

    def run_async(self, in_maps):
        concat_in = [
            np.concatenate([np.asarray(m[name]) for m in in_maps], axis=0)
            for name in self.in_names
        ]
        zeros = [
            np.zeros((self.n_cores * s[0], *s[1:]), d) for s, d in self.zero_shapes
        ]
        return self.fn(*concat_in, *zeros)

    def __call__(self, in_maps):
        out_arrs = self.run_async(in_maps)
        nc_ = self.n_cores
        return [
            {
                name: np.asarray(out_arrs[i]).reshape(
                    nc_, *self.out_avals[i].shape
                )[c]
                for i, name in enumerate(self.out_names)
            }
            for c in range(nc_)
        ]


def _get_compiled(edge_index, batch):
    import hashlib

    hsh = hashlib.sha256()
    hsh.update(np.ascontiguousarray(edge_index).tobytes())
    hsh.update(np.ascontiguousarray(batch).tobytes())
    key = hsh.hexdigest()
    if key not in _CACHE:
        pp = _preprocess(edge_index, batch)
        nc = _build_program(pp)
        runner = _Runner(nc, NCORES)
        _CACHE.clear()
        _CACHE[key] = (pp, runner)
    return _CACHE[key]


def _input_maps(pp, x, weights):
    """weights = (H1_w, H1_b, W1_w, W1_b, H2_w, H2_b, W2_w, W2_b)"""
    H1_w, H1_b, W1_w, W1_b, H2_w, H2_b, W2_w, W2_b = [
        np.asarray(a, np.float32) for a in weights
    ]
    iota_np = np.tile(np.arange(128, dtype=np.float32), (128, 1))
    base = {"h": x, "iota": iota_np}
    for L, (Hw, Hb, Ww, Wb) in enumerate(
        [(H1_w, H1_b, W1_w, W1_b), (H2_w, H2_b, W2_w, W2_b)], start=1
    ):
        hwt = np.ascontiguousarray(Hw.T)
        base[f"hwt{L}"] = hwt
        base[f"hwt_ext{L}"] = np.vstack([hwt, Hb[None, :]])
        base[f"hb{L}"] = np.ascontiguousarray(Hb[:, None])
        base[f"wwt_ext{L}"] = np.vstack(
            [np.ascontiguousarray(Ww.T), Wb[None, :]]
        )
    return [
        {
            **base,
            "eidx": pp["eidx"][c],
            "dstl": pp["dstl"][c],
            "gloc": pp["gloc"][c],
        }
        for c in range(NCORES)
    ]


def kernel(
    x,
    H1_w,
    H1_b,
    W1_w,
    W1_b,
    H2_w,
    H2_b,
    W2_w,
    W2_b,
    edge_index,
    batch,
    num_graphs,
):
    x = np.ascontiguousarray(np.asarray(x, dtype=np.float32))
    num_graphs = int(num_graphs)
    pp, runner = _get_compiled(edge_index, batch)
    NW = pp["NW"]

    maps = _input_maps(
        pp, x, (H1_w, H1_b, W1_w, W1_b, H2_w, H2_b, W2_w, W2_b)
    )
    res = runner(maps)

    out = np.zeros((num_graphs, FP), np.float32)
    for c in range(NCORES):
        for w in range(NW):
            gb = int(pp["gbases"][c, w])
            rows = min(128, num_graphs - gb)
            out[gb : gb + rows] += res[c]["pooled"][w][:rows]
    return out


def benchmark(inputs, iters=5):
    """Time the on-device launch with inputs pre-placed on device
    (excludes host preprocessing + h2d; donated output zeros are created
    on-device). Returns per-iteration seconds."""
    import time
    import jax
    import jax.numpy as jnp
    from jax.sharding import Mesh, NamedSharding, PartitionSpec

    x = np.ascontiguousarray(np.asarray(inputs["x"], np.float32))
    pp, runner = _get_compiled(inputs["edge_index"], inputs["batch"])
    weights = tuple(
        inputs[k]
        for k in ("H1_w", "H1_b", "W1_w", "W1_b", "H2_w", "H2_b", "W2_w", "W2_b")
    )
    maps = _input_maps(pp, x, weights)

    devices = jax.devices()[:NCORES]
    mesh = Mesh(np.asarray(devices), ("core",))
    sh = NamedSharding(mesh, PartitionSpec("core"))
    din = [
        jax.device_put(
            np.concatenate([np.asarray(m[name]) for m in maps], axis=0), sh
        )
        for name in runner.in_names
    ]

    def dev_zeros():
        return [
            jax.device_put(jnp.zeros((NCORES * s[0], *s[1:]), d), sh)
            for s, d in runner.zero_shapes
        ]

    times = []
    for it in range(iters + 1):
        z = dev_zeros()
        jax.block_until_ready(z)
        t0 = time.perf_counter()
        o = runner.fn(*din, *z)
        jax.block_until_ready(o)
        t1 = time.perf_counter()
        if it > 0:
            times.append(t1 - t0)
    return times


# revision 16
# speedup vs baseline: 95.7723x; 1.0254x over previous
"""NeuralFP GNN message-passing kernel for 8 Trainium2 NeuronCores.

Strategy (graph-level data parallel, per sharding hint):
  - Nodes are partitioned into 8 contiguous ranges of 6250; each core owns
    the aggregation + MLP + softmax + graph pooling for its node range.
  - Incident edges are bucketed on host by (owner core, 128-node dst tile,
    src half-block) and gathered on device with gpsimd dma_gather (int16
    indices limit 32767 -> two 25000-row source blocks), spread over 4
    SWDGE queues for descriptor-level parallelism.
  - segment_sum over edges: per 128-edge block, a 0/1 "onehot" matrix
    (built on DVE via is_equal against an iota table) reduces gathered
    rows into the 128 dst slots through a PE matmul accumulated in PSUM.
  - '+ h' self-loop term is folded in as explicit (v, v) edges.
  - softmax: logits are bounded (|logit| < ~8), so exp without max-shift;
    row sums come free via the ACT accumulate port; the 1/sum scaling is
    folded into the pooling matmul's onehot weights.
  - pooling: batch indices are sorted, so node tiles are grouped into <=128
    graph windows; pooling matmuls accumulate a [128, 2048] PSUM window
    across tiles; layer-1 windows park in SBUF, layer-2 adds and flushes.
  - BOTH layers run in ONE launch: layer 1 writes its h1 slice to internal
    DRAM, an on-device AllGather replicates h1 across the 8 cores, layer 2
    gathers from the replicated copy. Host overlap-adds the per-core
    per-window pooled outputs into the [num_graphs, 2048] result.
"""

import os
import numpy as np

N = 50000
F = 64
FP = 2048
NCORES = 8
NPC = N // NCORES  # 6250 nodes per core
TN = 128  # node tile
NT = (NPC + TN - 1) // TN  # 49 node tiles per core
NPAIR = (NT + 1) // 2  # 25 tile pairs (last pair has 1 tile)
BLK = 25000  # src index block (int16 limit)
FPC = 512  # fp chunk (1 PSUM bank)
NCH = FP // FPC  # 4 chunks

_CACHE = {}


def _roundup(x, m):
    return ((x + m - 1) // m) * m


def _pair_tiles(p):
    return [t for t in (2 * p, 2 * p + 1) if t < NT]


def _preprocess(edge_index, batch):
    """Bucket edges and build all per-core device tables."""
    src = np.asarray(edge_index[0], dtype=np.int64)
    dst = np.asarray(edge_index[1], dtype=np.int64)
    loop = np.arange(N, dtype=np.int64)  # self loops implement '+ h'
    src = np.concatenate([src, loop])
    dst = np.concatenate([dst, loop])
    batch = np.asarray(batch, dtype=np.int64)

    core = dst // NPC
    dst_local = dst - core * NPC
    t = dst_local // TN
    b = src // BLK
    d_in_tile = dst_local % TN
    idx_local = (src - b * BLK).astype(np.int64)

    # bucket edge lists per (core, t, b); sort by src idx within each bucket
    # (HBM page locality for the gather descriptors)
    key = ((core * NT + t) * 2 + b).astype(np.int64)
    order = np.lexsort((idx_local, key))
    key_s = key[order]
    idx_s = idx_local[order]
    dit_s = d_in_tile[order]
    counts = np.bincount(key_s, minlength=NCORES * NT * 2).reshape(NCORES, NT, 2)
    starts = np.zeros(NCORES * NT * 2 + 1, np.int64)
    np.cumsum(counts.reshape(-1), out=starts[1:])

    caps = np.maximum(
        _roundup(counts.max(axis=0), TN), TN
    )  # [NT, 2] compile-time bucket capacities

    # per-pair layout: segments in order [t0b0, t1b0, t0b1, t1b1]
    seg_col = np.zeros((NT, 2), np.int64)
    pair_cols = np.zeros(NPAIR, np.int64)
    pair_col_off = np.zeros(NPAIR, np.int64)
    col = 0
    for p in range(NPAIR):
        pair_col_off[p] = col
        for bb in range(2):
            for tt in _pair_tiles(p):
                seg_col[tt, bb] = col
                col += int(caps[tt, bb]) // TN
        pair_cols[p] = col - pair_col_off[p]
    total_cols = col

    ioff = np.zeros((NT, 2), np.int64)
    io = 0
    for p in range(NPAIR):
        for bb in range(2):
            for tt in _pair_tiles(p):
                ioff[tt, bb] = io
                io += int(caps[tt, bb]) // 16
    total_icols = io

    eidx = np.zeros((NCORES, 128, total_icols), np.int16)
    dstl = -np.ones((NCORES, 128, total_cols), np.float32)

    for c in range(NCORES):
        for ti in range(NT):
            for bb in range(2):
                k = (c * NT + ti) * 2 + bb
                s, e = starts[k], starts[k] + counts[c, ti, bb]
                cap = int(caps[ti, bb])
                vals = np.zeros(cap, np.int16)
                vals[: e - s] = idx_s[s:e]
                seg = vals.reshape(cap // 16, 16).T  # [16, cols]
                io = int(ioff[ti, bb])
                eidx[c, :, io : io + cap // 16] = np.tile(seg, (8, 1))
                dv = -np.ones(cap, np.float32)
                dv[: e - s] = dit_s[s:e]
                col0 = int(seg_col[ti, bb])
                dstl[c, :, col0 : col0 + cap // TN] = dv.reshape(cap // TN, TN).T

    windows = _common_windows(batch)
    NW = len(windows)
    gloc = -np.ones((NCORES, 128, NT), np.float32)
    gbases = np.zeros((NCORES, NW), np.int64)
    for c in range(NCORES):
        ng = batch[c * NPC : (c + 1) * NPC]
        for w, tl in enumerate(windows):
            gb = int(ng[tl[0] * TN])
            gbases[c, w] = gb
            for ti in tl:
                lo = ti * TN
                hi = min((ti + 1) * TN, NPC)
                span = ng[lo:hi] - gb
                assert span.max() < 128 and span.min() >= 0
                gloc[c, : hi - lo, ti] = span

    return dict(
        caps=caps,
        seg_col=seg_col,
        pair_col_off=pair_col_off,
        pair_cols=pair_cols,
        ioff=ioff,
        total_cols=total_cols,
        total_icols=total_icols,
        eidx=eidx,
        dstl=dstl,
        gloc=gloc,
        gbases=gbases,
        windows=windows,
        NW=NW,
    )


def _common_windows(batch):
    """Split the NT node tiles into windows (aligned to tile PAIRS) with
    graph span <= 128 on EVERY core simultaneously."""
    batch = np.asarray(batch, dtype=np.int64)
    spans = np.zeros((NCORES, NT, 2), np.int64)
    for c in range(NCORES):
        ng = batch[c * NPC : (c + 1) * NPC]
        for ti in range(NT):
            lo = ti * TN
            hi = min((ti + 1) * TN, NPC)
            spans[c, ti] = (ng[lo], ng[hi - 1])
    wins = []
    cur = [0, 1]
    for p in range(1, NPAIR):
        tl = _pair_tiles(p)
        ok = all(
            spans[c, tl[-1], 1] - spans[c, cur[0], 0] < 128 for c in range(NCORES)
        )
        if ok:
            cur.extend(tl)
        else:
            wins.append(cur)
            cur = list(tl)
    wins.append(cur)
    return wins


def _build_program(pp, reps=1, cfg=None):
    import concourse.bacc as bacc
    import concourse.mybir as mybir
    import concourse.tile as tile

    caps = pp["caps"]
    seg_col = pp["seg_col"]
    ioff = pp["ioff"]
    total_cols = pp["total_cols"]
    total_icols = pp["total_icols"]
    windows = pp["windows"]
    NW = pp["NW"]
    pair_cols = pp["pair_cols"]
    pair_col_off = pp["pair_col_off"]
    CT_MAX = int(pair_cols.max())
    cfg = dict(
        dict(gath_bufs=4, oh_bufs=8, sb_bufs=4, exp_bufs=3, pw_bufs=2,
             pss_bufs=3, fp_bufs=1, gath_gran="tile", pool_bf16=True),
        **(cfg or {}),
    )

    nc = bacc.Bacc(
        "TRN2", target_bir_lowering=False, debug=False, num_swdge_queues=4
    )
    f32 = mybir.dt.float32
    h = nc.dram_tensor("h", [N, F], f32, kind="ExternalInput")
    wts = []
    for L in (1, 2):
        wts.append(
            dict(
                hwt=nc.dram_tensor(f"hwt{L}", [F, F], f32, kind="ExternalInput"),
                hwt_ext=nc.dram_tensor(
                    f"hwt_ext{L}", [F + 1, F], f32, kind="ExternalInput"
                ),
                hb=nc.dram_tensor(f"hb{L}", [F, 1], f32, kind="ExternalInput"),
                wwt_ext=nc.dram_tensor(
                    f"wwt_ext{L}", [F + 1, FP], f32, kind="ExternalInput"
                ),
            )
        )
    iota = nc.dram_tensor("iota", [128, 128], f32, kind="ExternalInput")
    eidx = nc.dram_tensor(
        "eidx", [128, total_icols], mybir.dt.int16, kind="ExternalInput"
    )
    dstl = nc.dram_tensor("dstl", [128, total_cols], f32, kind="ExternalInput")
    gloc = nc.dram_tensor("gloc", [128, NT], f32, kind="ExternalInput")
    pooled = nc.dram_tensor("pooled", [NW, 128, FP], f32, kind="ExternalOutput")
    h1_mine = nc.dram_tensor("h1_mine", [NPC, F], f32)
    h1_all = nc.dram_tensor("h1_all", [N, F], f32, addr_space="Shared")

    with tile.TileContext(nc) as tc:
        with (
            tc.tile_pool(name="const", bufs=1) as cpool,
            tc.tile_pool(name="gath", bufs=cfg["gath_bufs"]) as gpool,
            tc.tile_pool(name="oh", bufs=cfg["oh_bufs"]) as ohpool,
            tc.tile_pool(name="sb", bufs=cfg["sb_bufs"]) as sbpool,
            tc.tile_pool(name="exp", bufs=cfg["exp_bufs"]) as epool,
            tc.tile_pool(name="pw", bufs=cfg["pw_bufs"]) as pwpool,
            tc.tile_pool(name="ps_small", bufs=cfg["pss_bufs"], space="PSUM") as pss,
            tc.tile_pool(name="ps_fp", bufs=cfg["fp_bufs"], space="PSUM") as psf,
            tc.tile_pool(name="ps_win", bufs=1, space="PSUM") as psw,
        ):
            eidx_sb = cpool.tile([128, total_icols], mybir.dt.int16)
            nc.sync.dma_start(out=eidx_sb[:], in_=eidx[:])
            dstl_sb = cpool.tile([128, total_cols], f32)
            nc.sync.dma_start(out=dstl_sb[:], in_=dstl[:])
            gloc_sb = cpool.tile([128, NT], f32)
            nc.sync.dma_start(out=gloc_sb[:], in_=gloc[:])
            iota_sb = cpool.tile([128, 128], f32)
            nc.sync.dma_start(out=iota_sb[:], in_=iota[:])
            wsb = []
            for L in (0, 1):
                d = {}
                d["hwt"] = cpool.tile([F, F], f32, tag=f"hwt{L}", name=f"hwt{L}_sb")
                nc.sync.dma_start(out=d["hwt"][:], in_=wts[L]["hwt"][:])
                d["hwt_ext"] = cpool.tile([F + 1, F], f32, tag=f"hwte{L}", name=f"hwte{L}_sb")
                nc.sync.dma_start(out=d["hwt_ext"][:], in_=wts[L]["hwt_ext"][:])
                d["hb"] = cpool.tile([F, 1], f32, tag=f"hb{L}", name=f"hb{L}_sb")
                nc.sync.dma_start(out=d["hb"][:], in_=wts[L]["hb"][:])
                d["wwt"] = cpool.tile([F + 1, FP], f32, tag=f"wwt{L}", name=f"wwt{L}_sb")
                nc.sync.dma_start(out=d["wwt"][:], in_=wts[L]["wwt_ext"][:])
                wsb.append(d)
            pw_acc = [
                cpool.tile([128, FP], f32, tag=f"pwacc{w}", name=f"pwacc{w}_sb") for w in range(NW)
            ]

            qctr = [0]
            gath_tiles = {}

            def emit_gathers(p, src_dram):
                if p in gath_tiles:
                    return gath_tiles[p]
                base = int(pair_col_off[p])
                gt = gpool.tile([128, CT_MAX, F], f32, tag="gath")
                tl = _pair_tiles(p)
                for bb in range(2):
                    groups = (
                        [[tt] for tt in tl] if cfg["gath_gran"] == "tile" else [tl]
                    )
                    for grp in groups:
                        cap = sum(int(caps[tt, bb]) for tt in grp)
                        io = int(ioff[grp[0], bb])
                        colo = int(seg_col[grp[0], bb]) - base
                        nc.gpsimd.dma_gather(
                            gt[:, colo : colo + cap // TN, :],
                            src_dram[bb * BLK : (bb + 1) * BLK, :],
                            eidx_sb[:, io : io + cap // 16],
                            cap,
                            cap,
                            F,
                            elem_step=F,
                            single_packet=False,
                            queue_num=qctr[0] % 4,
                        )
                        qctr[0] += 1
                gath_tiles.clear()
                gath_tiles[p] = (gt, base)
                return gath_tiles[p]

            for _rep in range(reps):
                for L in (0, 1):
                    W = wsb[L]
                    src_dram = h if L == 0 else h1_all
                    gath_tiles.clear()
                    for w, tl_w in enumerate(windows):
                        psum_win = psw.tile([128, FP], f32)
                        for wi, t in enumerate(tl_w):
                            gt, gbase = emit_gathers(t // 2, src_dram)
                            bcols = []
                            for bb in range(2):
                                c0 = int(seg_col[t, bb]) - gbase
                                bcols.extend(
                                    range(c0, c0 + int(caps[t, bb]) // TN)
                                )
                            psum_aggT = pss.tile([F, 128], f32, tag="pss")
                            for bi, blk in enumerate(bcols):
                                oh = ohpool.tile([128, 128], f32, tag="oh")
                                dc = gbase + blk
                                nc.vector.tensor_tensor(
                                    out=oh[:],
                                    in0=dstl_sb[:, dc : dc + 1].to_broadcast(
                                        [128, 128]
                                    ),
                                    in1=iota_sb[:],
                                    op=mybir.AluOpType.is_equal,
                                )
                                nc.tensor.matmul(
                                    psum_aggT[:],
                                    lhsT=gt[:, blk, :],
                                    rhs=oh[:],
                                    start=(bi == 0),
                                    stop=(bi == len(bcols) - 1),
                                )
                            aggT_ext = sbpool.tile([F + 1, 128], f32, tag="aggT")
                            nc.vector.memset(aggT_ext[F : F + 1, :], 1.0)
                            nc.scalar.copy(aggT_ext[0:F, :], psum_aggT[:])
                            # updT = sigmoid(Hw @ aggT + hb)
                            psum_updT = pss.tile([F, 128], f32, tag="pss")
                            nc.tensor.matmul(
                                psum_updT[:],
                                lhsT=W["hwt"][:],
                                rhs=aggT_ext[0:F, :],
                                start=True,
                                stop=True,
                            )
                            updT_ext = sbpool.tile([F + 1, 128], f32, tag="updT")
                            nc.vector.memset(updT_ext[F : F + 1, :], 1.0)
                            nc.scalar.activation(
                                updT_ext[0:F, :],
                                psum_updT[:],
                                mybir.ActivationFunctionType.Sigmoid,
                                bias=W["hb"][:],
                            )
                            if L == 0:
                                # row-major upd for the h1 exchange
                                psum_upd = pss.tile([128, F], f32, tag="pss")
                                nc.tensor.matmul(
                                    psum_upd[:],
                                    lhsT=aggT_ext[:],
                                    rhs=W["hwt_ext"][:],
                                    start=True,
                                    stop=True,
                                )
                                upd_sb = sbpool.tile([128, F], f32, tag="upd")
                                nc.scalar.activation(
                                    upd_sb[:],
                                    psum_upd[:],
                                    mybir.ActivationFunctionType.Sigmoid,
                                )
                                rows = min(TN, NPC - t * TN)
                                nc.sync.dma_start(
                                    out=h1_mine[t * TN : t * TN + rows, :],
                                    in_=upd_sb[0:rows, :],
                                )
                            expdt = (
                                mybir.dt.bfloat16 if cfg["pool_bf16"] else f32
                            )
                            exp_sb = epool.tile([128, FP], expdt, tag="exp")
                            rs4 = sbpool.tile([128, NCH], f32, tag="rs4")
                            for ch in range(NCH):
                                psum_fp = psf.tile([128, FPC], f32, tag="fp")
                                nc.tensor.matmul(
                                    psum_fp[:],
                                    lhsT=updT_ext[:],
                                    rhs=W["wwt"][:, ch * FPC : (ch + 1) * FPC],
                                    start=True,
                                    stop=True,
                                )
                                nc.scalar.activation(
                                    exp_sb[:, ch * FPC : (ch + 1) * FPC],
                                    psum_fp[:],
                                    mybir.ActivationFunctionType.Exp,
                                    accum_out=rs4[:, ch : ch + 1],
                                )
                            rs = sbpool.tile([128, 1], f32, tag="rs")
                            nc.vector.tensor_reduce(
                                rs[:],
                                rs4[:],
                                mybir.AxisListType.X,
                                mybir.AluOpType.add,
                            )
                            recip = sbpool.tile([128, 1], f32, tag="recip")
                            nc.vector.reciprocal(recip[:], rs[:])
                            woh = ohpool.tile([128, 128], expdt, tag="woh")
                            nc.vector.tensor_tensor(
                                out=woh[:],
                                in0=gloc_sb[:, t : t + 1].to_broadcast([128, 128]),
                                in1=iota_sb[:],
                                op=mybir.AluOpType.is_equal,
                            )
                            nc.vector.tensor_scalar(
                                out=woh[:],
                                in0=woh[:],
                                scalar1=recip[:],
                                scalar2=None,
                                op0=mybir.AluOpType.mult,
                            )
                            for ch in range(NCH):
                                nc.tensor.matmul(
                                    psum_win[:, ch * FPC : (ch + 1) * FPC],
                                    lhsT=woh[:],
                                    rhs=exp_sb[:, ch * FPC : (ch + 1) * FPC],
                                    start=(wi == 0),
                                    stop=(wi == len(tl_w) - 1),
                                )
                        if L == 0:
                            nc.scalar.copy(pw_acc[w][:], psum_win[:])
                        else:
                            pw_sb = pwpool.tile([128, FP], f32, tag="pw")
                            nc.vector.tensor_tensor(
                                out=pw_sb[:],
                                in0=pw_acc[w][:],
                                in1=psum_win[:],
                                op=mybir.AluOpType.add,
                            )
                            nc.sync.dma_start(out=pooled[w], in_=pw_sb[:])
                    if L == 0:
                        nc.gpsimd.collective_compute(
                            "AllGather",
                            mybir.AluOpType.bypass,
                            replica_groups=[list(range(NCORES))],
                            ins=[h1_mine[:]],
                            outs=[h1_all[:]],
                        )

    nc.compile()
    return nc


class _Runner:
    """Cached shard_map executor for one compiled Bass program (replicates
    bass2jax.run_bass_via_pjrt but builds the jitted callable once)."""

    def __init__(self, nc, n_cores):
        import jax
        import concourse.mybir as mybir
        from concourse import bass2jax
        from jax.sharding import Mesh, PartitionSpec
        from jax.experimental.shard_map import shard_map

        bass2jax.install_neuronx_cc_hook()
        self.nc = nc
        self.n_cores = n_cores
        partition_name = (
            nc.partition_id_tensor.name if nc.partition_id_tensor else None
        )
        in_names, out_names, out_avals, zero_shapes = [], [], [], []
        for alloc in nc.m.functions[0].allocations:
            if not isinstance(alloc, mybir.MemoryLocationSet):
                continue
            name = alloc.memorylocations[0].name
            if alloc.kind == "ExternalInput":
                if name != partition_name:
                    in_names.append(name)
            elif alloc.kind == "ExternalOutput":
                shape = tuple(alloc.tensor_shape)
                dtype = mybir.dt.np(alloc.dtype)
                out_names.append(name)
                out_avals.append(jax.core.ShapedArray(shape, dtype))
                zero_shapes.append((shape, dtype))
        self.in_names = in_names
        self.out_names = out_names
        self.zero_shapes = zero_shapes
        self.out_avals = out_avals
        n_params = len(in_names)
        n_outs = len(out_avals)
        all_in_names = list(in_names) + list(out_names)
        if partition_name is not None:
            all_in_names.append(partition_name)

        def _body(*args):
            operands = list(args)
            if partition_name is not None:
                operands.append(bass2jax.partition_id_tensor())
            outs = bass2jax._bass_exec_p.bind(
                *operands,
                out_avals=tuple(out_avals),
                in_names=tuple(all_in_names),
                out_names=tuple(out_names),
                lowering_input_output_aliases=(),
                sim_require_finite=True,
                sim_require_nnan=True,
                nc=nc,
            )
            return tuple(outs)

        devices = jax.devices()[:n_cores]
        assert len(devices) == n_cores
        mesh = Mesh(np.asarray(devices), ("core",))
        in_specs = (PartitionSpec("core"),) * (n_params + n_outs)
        out_specs = (PartitionSpec("core"),) * n_outs
        donate = tuple(range(n_params, n_params + n_outs))
        self.fn = jax.jit(
            shard_map(
                _body, mesh=mesh, in_specs=in_specs, out_specs=out_specs,
                check_rep=False,
            ),
            donate_argnums=donate,
            keep_unused=True,
        )

    def run_async(self, in_maps):
        concat_in = [
            np.concatenate([np.asarray(m[name]) for m in in_maps], axis=0)
            for name in self.in_names
        ]
        zeros = [
            np.zeros((self.n_cores * s[0], *s[1:]), d) for s, d in self.zero_shapes
        ]
        return self.fn(*concat_in, *zeros)

    def __call__(self, in_maps):
        out_arrs = self.run_async(in_maps)
        nc_ = self.n_cores
        return [
            {
                name: np.asarray(out_arrs[i]).reshape(
                    nc_, *self.out_avals[i].shape
                )[c]
                for i, name in enumerate(self.out_names)
            }
            for c in range(nc_)
        ]


def _get_compiled(edge_index, batch):
    import hashlib

    hsh = hashlib.sha256()
    hsh.update(np.ascontiguousarray(edge_index).tobytes())
    hsh.update(np.ascontiguousarray(batch).tobytes())
    key = hsh.hexdigest()
    if key not in _CACHE:
        pp = _preprocess(edge_index, batch)
        nc = _build_program(pp)
        runner = _Runner(nc, NCORES)
        _CACHE.clear()
        _CACHE[key] = (pp, runner)
    return _CACHE[key]


def _input_maps(pp, x, weights):
    """weights = (H1_w, H1_b, W1_w, W1_b, H2_w, H2_b, W2_w, W2_b)"""
    H1_w, H1_b, W1_w, W1_b, H2_w, H2_b, W2_w, W2_b = [
        np.asarray(a, np.float32) for a in weights
    ]
    iota_np = np.tile(np.arange(128, dtype=np.float32), (128, 1))
    base = {"h": x, "iota": iota_np}
    for L, (Hw, Hb, Ww, Wb) in enumerate(
        [(H1_w, H1_b, W1_w, W1_b), (H2_w, H2_b, W2_w, W2_b)], start=1
    ):
        hwt = np.ascontiguousarray(Hw.T)
        base[f"hwt{L}"] = hwt
        base[f"hwt_ext{L}"] = np.vstack([hwt, Hb[None, :]])
        base[f"hb{L}"] = np.ascontiguousarray(Hb[:, None])
        base[f"wwt_ext{L}"] = np.vstack(
            [np.ascontiguousarray(Ww.T), Wb[None, :]]
        )
    return [
        {
            **base,
            "eidx": pp["eidx"][c],
            "dstl": pp["dstl"][c],
            "gloc": pp["gloc"][c],
        }
        for c in range(NCORES)
    ]


def kernel(
    x,
    H1_w,
    H1_b,
    W1_w,
    W1_b,
    H2_w,
    H2_b,
    W2_w,
    W2_b,
    edge_index,
    batch,
    num_graphs,
):
    x = np.ascontiguousarray(np.asarray(x, dtype=np.float32))
    num_graphs = int(num_graphs)
    pp, runner = _get_compiled(edge_index, batch)
    NW = pp["NW"]

    maps = _input_maps(
        pp, x, (H1_w, H1_b, W1_w, W1_b, H2_w, H2_b, W2_w, W2_b)
    )
    res = runner(maps)

    out = np.zeros((num_graphs, FP), np.float32)
    for c in range(NCORES):
        for w in range(NW):
            gb = int(pp["gbases"][c, w])
            rows = min(128, num_graphs - gb)
            out[gb : gb + rows] += res[c]["pooled"][w][:rows]
    return out


def benchmark(inputs, iters=5):
    """Time the on-device launch with inputs pre-placed on device
    (excludes host preprocessing + h2d; donated output zeros are created
    on-device). Returns per-iteration seconds."""
    import time
    import jax
    import jax.numpy as jnp
    from jax.sharding import Mesh, NamedSharding, PartitionSpec

    x = np.ascontiguousarray(np.asarray(inputs["x"], np.float32))
    pp, runner = _get_compiled(inputs["edge_index"], inputs["batch"])
    weights = tuple(
        inputs[k]
        for k in ("H1_w", "H1_b", "W1_w", "W1_b", "H2_w", "H2_b", "W2_w", "W2_b")
    )
    maps = _input_maps(pp, x, weights)

    devices = jax.devices()[:NCORES]
    mesh = Mesh(np.asarray(devices), ("core",))
    sh = NamedSharding(mesh, PartitionSpec("core"))
    din = [
        jax.device_put(
            np.concatenate([np.asarray(m[name]) for m in maps], axis=0), sh
        )
        for name in runner.in_names
    ]

    def dev_zeros():
        return [
            jax.device_put(jnp.zeros((NCORES * s[0], *s[1:]), d), sh)
            for s, d in runner.zero_shapes
        ]

    times = []
    for it in range(iters + 1):
        z = dev_zeros()
        jax.block_until_ready(z)
        t0 = time.perf_counter()
        o = runner.fn(*din, *z)
        jax.block_until_ready(o)
        t1 = time.perf_counter()
        if it > 0:
            times.append(t1 - t0)
    return times
